# revision 1
# baseline (speedup 1.0000x reference)
"""CrossAttnBlock kernel for 8 Trainium2 NeuronCores.

Sharding: core c -> (batch b = c//2, token-half s = c%2), 512 query tokens
per core. Cross-attention K/V is computed fully per core (duplicated within
the pair); after cross-attention the per-core residual x2 is exchanged with
one 8-rank AllGather so each core rebuilds the partner half's self-attn K/V
locally (attention is permutation-invariant over KV tokens, so own tokens
always sit at positions 0:512).

All activations are feature-major ([feature, token]) so every linear layer
consumes natural-layout weights as the stationary matmul operand and no
on-device transposes are needed. Matmuls run in float32r (fp32 storage,
~tf32 matmul precision, 1 cycle/row at N=512). Softmax skips the max
subtraction (scores are O(1) for this problem) and gets sum-exp for free
from a ones-column appended to V. K/V are spilled to DRAM and streamed back
per head to fit SBUF.
"""
import sys

sys.path.insert(0, '/opt/trn_rl_repo')

import numpy as np
import concourse.bass as bass
from concourse import bacc
import concourse.tile as tile
from concourse import mybir

F32R = mybir.dt.float32r
F32 = mybir.dt.float32
AF = mybir.ActivationFunctionType
OP = mybir.AluOpType

N_CORES = 8
B, NSEQ, D, H, HD = 4, 1024, 1024, 16, 64
T = 512            # tokens owned per core
TF = 1024          # full token count per batch
C8 = D // 128      # feature chunks
SCALE = 1.0 / float(np.sqrt(np.float32(HD)))
EPS = 1e-6

_PROGRAM_CACHE = {}


def _rearr_w(w):
    """[Din, N] dram AP -> [128, Din//128, N] (partition, chunk, col)."""
    return w.rearrange("(c p) n -> p c n", p=128)


def _build_program():
    nc = bacc.Bacc("TRN2", target_bir_lowering=False, debug=False,
                   num_devices=N_CORES)

    dp = {}
    dp["xT"] = nc.declare_dram_parameter("xT", [D, T], F32R, isOutput=False)
    dp["kvT"] = nc.declare_dram_parameter("kvT", [D, TF], F32R, isOutput=False)
    for nm, sh in [("wq", [D, D]), ("wkv", [D, 2 * D]), ("wqkv", [D, 3 * D]),
                   ("wco", [D, D]), ("wso", [D, D]), ("w1", [D, 4 * D]),
                   ("w2", [4 * D, D])]:
        dp[nm] = nc.declare_dram_parameter(nm, sh, F32R, isOutput=False)
    for i in (1, 2, 3, 4):
        for sb in ("s", "b"):
            dp[f"ln{i}_{sb}"] = nc.declare_dram_parameter(
                f"ln{i}_{sb}", [1, D], F32, isOutput=False)
    for nm, n in [("bco", D), ("bso", D), ("b1", 4 * D), ("b2", D)]:
        dp[nm] = nc.declare_dram_parameter(nm, [1, n], F32, isOutput=False)
    dp["ones"] = nc.declare_dram_parameter("ones", [128, 128], F32R,
                                           isOutput=False)
    dp["outT"] = nc.declare_dram_parameter("outT", [D, T], F32R, isOutput=True)

    with tile.TileContext(nc) as tc:
        _emit(nc, tc, dp)
    nc.compile()
    return nc


def _emit(nc, tc, dp):
    import contextlib

    ctx = contextlib.ExitStack()
    with ctx:
        consts = ctx.enter_context(tc.tile_pool(name="consts", bufs=1))
        outer = ctx.enter_context(tc.tile_pool(name="outer", bufs=1))
        pp = ctx.enter_context(tc.tile_pool(name="pp", bufs=1, space="PSUM"))
        small = ctx.enter_context(tc.tile_pool(name="small", bufs=1))
        dramp = ctx.enter_context(tc.tile_pool(name="dramp", bufs=1,
                                               space="DRAM"))

        # ---------- constants ----------
        ones_sb = consts.tile([128, 128], F32R)
        nc.sync.dma_start(out=ones_sb[:], in_=dp["ones"][:])
        ones_col = ones_sb[:, 0:1]
        ones_row = ones_sb[0:1, :]
        eps_t = consts.tile([1, 1], F32)
        nc.vector.memset(eps_t[:], EPS)

        def load_col(name, nchunk):
            col = consts.tile([128, nchunk], F32, name=f"col_{name}")
            nc.sync.dma_start(
                out=col[:], in_=dp[name].rearrange("o (c p) -> p (o c)", p=128))
            return col

        ln_c = {f"{i}{sb}": load_col(f"ln{i}_{sb}", C8)
                for i in (1, 2, 3, 4) for sb in ("s", "b")}
        bco_c = load_col("bco", C8)
        bso_c = load_col("bso", C8)
        b1_c = load_col("b1", 32)
        b2_c = load_col("b2", C8)

        pid = nc.sync.partition_id()
        partner = (pid // 2) * 2 + (1 - pid % 2)

        # ---------- DRAM intermediates ----------
        x2_d = dramp.tile([128, C8, T], F32R, name="x2_d")          # own x2
        ag_out = dramp.tile([N_CORES, 128, C8 * T], F32R,
                            addr_space="Shared", name="ag_out")
        kT_d = dramp.tile([C8, 128, TF], F32R, name="kT_d")
        v_d = dramp.tile([128, 8, H, 65], F32R, name="v_d")
        kT2_d = dramp.tile([C8, 128, TF], F32R, name="kT2_d")
        v2_d = dramp.tile([128, 8, H, 65], F32R, name="v2_d")

        # ---------- generic helpers ----------
        def layer_norm(src_fn, dst, dst_sl, s_col, b_col, pool):
            """LN over the feature axis for 512 tokens.

            src_fn(c) -> [128, 512] fp32r AP (may DMA into a stream tile).
            dst: [128, C8, *] SBUF tile, dst_sl a 512-token slice.
            """
            stats_x = pp.tile([1, 512], F32, tag="sps", bufs=2, name="stats_x")
            stats_q = pp.tile([1, 512], F32, tag="ops", bufs=3, name="stats_q")
            for c in range(C8):
                xc = src_fn(c)
                sq = pool.tile([128, 512], F32R, tag="pt", bufs=3, name="sq")
                nc.scalar.activation(out=sq[:], in_=xc, func=AF.Square)
                nc.tensor.matmul(stats_x[:], ones_col, xc,
                                 start=(c == 0), stop=(c == C8 - 1),
                                 skip_group_check=True)
                nc.tensor.matmul(stats_q[:], ones_col, sq[:],
                                 start=(c == 0), stop=(c == C8 - 1),
                                 skip_group_check=True)
            mean = small.tile([1, 512], F32, tag="mean", bufs=1, name="mean")
            nc.vector.tensor_scalar_mul(mean[:], stats_x[:], 1.0 / D)
            var = small.tile([1, 512], F32, tag="var", bufs=1, name="var")
            nc.vector.tensor_scalar_mul(var[:], stats_q[:], 1.0 / D)
            m2 = small.tile([1, 512], F32, tag="m2", bufs=1, name="m2")
            nc.vector.tensor_mul(m2[:], mean[:], mean[:])
            nc.vector.tensor_sub(var[:], var[:], m2[:])
            std = small.tile([1, 512], F32, tag="std", bufs=1, name="std")
            nc.scalar.activation(out=std[:], in_=var[:], func=AF.Sqrt,
                                 bias=eps_t[:])
            inv = small.tile([1, 512], F32R, tag="inv", bufs=2, name="inv")
            with nc.allow_low_precision(reason="float32r is 32-bit"):
                nc.vector.reciprocal(inv[:], std[:])
            negminv = small.tile([1, 512], F32R, tag="negminv", bufs=2,
                                 name="negminv")
            nc.vector.tensor_mul(negminv[:], mean[:], inv[:])
            nc.vector.tensor_scalar_mul(negminv[:], negminv[:], -1.0)
            a0 = pp.tile([128, 512], F32, tag="mm", bufs=3, name="a0")
            nc.tensor.matmul(a0[:], ones_row, inv[:], start=True, stop=True)
            c0 = pp.tile([128, 512], F32, tag="mm", bufs=3, name="c0")
            nc.tensor.matmul(c0[:], ones_row, negminv[:], start=True, stop=True)
            for c in range(C8):
                xc = src_fn(c)
                nc.vector.tensor_mul(dst[:, c, dst_sl], xc, a0[:])
                nc.vector.tensor_add(dst[:, c, dst_sl], dst[:, c, dst_sl], c0[:])
                nc.vector.tensor_scalar(
                    dst[:, c, dst_sl], dst[:, c, dst_sl],
                    scalar1=s_col[:, c:c + 1], scalar2=b_col[:, c:c + 1],
                    op0=OP.mult, op1=OP.add)

        def dram_src(pool, dram_ap_fn, tag="lnsrc"):
            def src_fn(c):
                t = pool.tile([128, 512], F32R, tag=tag, bufs=2, name=tag)
                nc.sync.dma_start(out=t[:], in_=dram_ap_fn(c))
                return t[:]
            return src_fn

        def gemm_feat(w_dram, col_off, n_tiles, rhs_list, pool, evict, wtag="wst"):
            """Feature-major GEMM; rhs_list: [(rhs_fn(c) -> [128,512] AP, key)].
            evict(nt, key, psum_tile)."""
            wr = _rearr_w(w_dram)
            for nt in range(n_tiles):
                wt = pool.tile([128, C8, 128], F32R, tag=wtag, bufs=3,
                               name=f"w_{wtag}")
                nc.sync.dma_start(
                    out=wt[:],
                    in_=wr[:, :, col_off + nt * 128:col_off + (nt + 1) * 128])
                for (rhs_fn, key) in rhs_list:
                    ps = pp.tile([128, 512], F32, tag="mm", bufs=3, name="gps")
                    for c in range(C8):
                        nc.tensor.matmul(ps[:], wt[:, c, :], rhs_fn(c),
                                         start=(c == 0), stop=(c == C8 - 1))
                    evict(nt, key, ps)

        def spill(pool, ps_ap, dram_ap, tag="spill"):
            t = pool.tile([128, 512], F32R, tag=tag, bufs=2, name=tag)
            nc.vector.tensor_copy(out=t[:], in_=ps_ap)
            nc.sync.dma_start(out=dram_ap, in_=t[:])

        def build_v_group(pool, wv_dram_col0, src, jts, v_dram, jt_off):
            """Token-major V for a group of 128-token j-tiles, spilled to DRAM.

            wv_dram_col0: column offset of the V block inside its weight.
            src: [128, C8, 512] SBUF tile; jts: j-tile indices within src."""
            for dvh in range(2):
                wvh = pool.tile([128, C8, 512], F32R, tag="wvh", bufs=1,
                                name="wvh")
                nc.sync.dma_start(
                    out=wvh[:],
                    in_=wv_dram_col0[:, :, dvh * 512:(dvh + 1) * 512])
                for jt in jts:
                    ps = pp.tile([128, 512], F32, tag="mm", bufs=3, name="vps")
                    sl = slice((jt - jt_off) * 128, (jt - jt_off + 1) * 128)
                    for c in range(C8):
                        nc.tensor.matmul(ps[:], src[:, c, sl], wvh[:, c, :],
                                         start=(c == 0), stop=(c == C8 - 1))
                    t = pool.tile([128, 8, 64], F32R, tag="vspill", bufs=2,
                                  name="vspill")
                    nc.vector.tensor_copy(
                        out=t[:], in_=ps[:].rearrange("p (h e) -> p h e", h=8))
                    nc.sync.dma_start(
                        out=v_dram[:, jt, dvh * 8:(dvh + 1) * 8, 0:64], in_=t[:])
            for jt in jts:
                nc.sync.dma_start(out=v_dram[:, jt, :, 64], in_=ones_sb[:, 0:16])

        def attention(pool, qT, kT_dram, v_dram, oT):
            for h in range(H):
                ch, off = h // 2, (h % 2) * 64
                if off == 0:
                    kth = pool.tile([128, TF], F32R, tag="kth", bufs=2,
                                    name="kth")
                    nc.sync.dma_start(out=kth[:], in_=kT_dram[ch, :, :])
                vh = pool.tile([128, 8, 65], F32R, tag="vh", bufs=2, name="vh")
                nc.sync.dma_start(out=vh[:], in_=v_dram[:, :, h, :])
                o_ps = pp.tile([65, 512], F32, tag="ops", bufs=3, name="ops")
                for jt in range(8):
                    s_ps = pp.tile([128, 512], F32, tag="sps", bufs=2, name="sps")
                    nc.tensor.matmul(s_ps[:],
                                     kth[off:off + 64, jt * 128:(jt + 1) * 128],
                                     qT[off:off + 64, ch, :],
                                     start=True, stop=True)
                    pt = pool.tile([128, 512], F32R, tag="pt", bufs=3, name="pt")
                    nc.scalar.activation(out=pt[:], in_=s_ps[:], func=AF.Exp,
                                         scale=SCALE)
                    nc.tensor.matmul(o_ps[:], vh[:, jt, :], pt[:],
                                     start=(jt == 0), stop=(jt == 7),
                                     skip_group_check=True)
                zrec = small.tile([1, 512], F32, tag="zrec", bufs=2, name="zrec")
                nc.vector.reciprocal(zrec[:], o_ps[64:65, :])
                zd = dramp.tile([1, 512], F32, tag="zd", bufs=2, name="zd")
                nc.sync.dma_start(out=zd[:], in_=zrec[:])
                zb = pool.tile([64, 512], F32, tag="zb", bufs=2, name="zb")
                zsrc = bass.AP(tensor=zd.tensor, offset=zd.offset,
                               ap=[[0, 64]] + list(zd.ap[1:]))
                nc.sync.dma_start(out=zb[:], in_=zsrc)
                nc.vector.tensor_mul(oT[off:off + 64, ch, :], o_ps[0:64, :],
                                     zb[:])

        # ---------- load x ----------
        x1 = outer.tile([128, C8, T], F32R, tag="res512", bufs=2, name="x1")
        nc.sync.dma_start(out=x1[:],
                          in_=dp["xT"].rearrange("(c p) t -> p c t", p=128))

        # ================= Phase 1: cross-attention =================
        with tc.tile_pool(name="p1", bufs=1) as p1:
            q_in = p1.tile([128, C8, T], F32R, name="q_in")
            layer_norm(lambda c: x1[:, c, :], q_in, slice(0, 512),
                       ln_c["1s"], ln_c["1b"], p1)

            qT1 = p1.tile([128, C8, T], F32R, name="qT1")

            def ev_qT(nt, key, ps):
                nc.vector.tensor_copy(out=qT1[:, nt, :], in_=ps[:])

            gemm_feat(dp["wq"], 0, C8, [(lambda c: q_in[:, c, :], 0)], p1, ev_qT)

            kvT_r = dp["kvT"].rearrange("(c p) t -> p c t", p=128)
            kv_in = p1.tile([128, C8, TF], F32R, name="kv_in")

            def ev_kT(nt, th, ps):
                spill(p1, ps[:], kT_d[nt, :, th * 512:(th + 1) * 512])

            wv1 = _rearr_w(dp["wkv"])[:, :, D:2 * D]
            for th in range(2):
                sl = slice(th * 512, (th + 1) * 512)
                layer_norm(dram_src(p1, lambda c, sl=sl: kvT_r[:, c, sl]),
                           kv_in, sl, ln_c["2s"], ln_c["2b"], p1)
                gemm_feat(dp["wkv"], 0, C8,
                          [(lambda c, sl=sl: kv_in[:, c, sl], th)], p1, ev_kT)
                build_v_group(p1, wv1, kv_in[:, :, sl], range(th * 4, th * 4 + 4),
                              v_d, th * 4)

            oT1 = p1.tile([128, C8, T], F32R, name="oT1")
            attention(p1, qT1, kT_d, v_d, oT1)

            # x2 = x1 + Wco @ o + bco  (straight to DRAM; it is the AG input)
            def ev_x2(nt, key, ps):
                t = p1.tile([128, 512], F32R, tag="spill", bufs=2, name="x2s")
                nc.vector.scalar_tensor_tensor(
                    out=t[:], in0=ps[:], scalar=bco_c[:, nt:nt + 1],
                    in1=x1[:, nt, :], op0=OP.add, op1=OP.add)
                nc.sync.dma_start(out=x2_d[:, nt, :], in_=t[:])

            gemm_feat(dp["wco"], 0, C8, [(lambda c: oT1[:, c, :], 0)], p1, ev_x2)

        # ================= x2 exchange (8-rank AllGather) =================
        nc.gpsimd.collective_compute(
            "AllGather", OP.bypass,
            ins=[x2_d[:].rearrange("p c t -> p (c t)")],
            outs=[ag_out[:]],
            replica_groups=[list(range(N_CORES))])
        ag_rem = ag_out[bass.ds(partner, 1), :, :].rearrange(
            "o p (c t) -> p (o c) t", c=C8)

        # ================= Phase 2: self-attention =================
        with tc.tile_pool(name="p2", bufs=1) as p2:
            s_own = p2.tile([128, C8, T], F32R, name="s_own")
            layer_norm(dram_src(p2, lambda c: x2_d[:, c, :]), s_own,
                       slice(0, 512), ln_c["3s"], ln_c["3b"], p2)

            qT2 = p2.tile([128, C8, T], F32R, name="qT2")

            def ev_qT2(nt, key, ps):
                nc.vector.tensor_copy(out=qT2[:, nt, :], in_=ps[:])

            gemm_feat(dp["wqkv"], 0, C8, [(lambda c: s_own[:, c, :], 0)], p2,
                      ev_qT2)

            def ev_kT2(nt, half, ps):
                spill(p2, ps[:], kT2_d[nt, :, half * 512:(half + 1) * 512])

            gemm_feat(dp["wqkv"], D, C8, [(lambda c: s_own[:, c, :], 0)], p2,
                      ev_kT2)

            wv2 = _rearr_w(dp["wqkv"])[:, :, 2 * D:3 * D]
            build_v_group(p2, wv2, s_own, range(0, 4), v2_d, 0)

            # remote half (depends on the AllGather)
            s_rem = p2.tile([128, C8, T], F32R, name="s_rem")
            layer_norm(dram_src(p2, lambda c: ag_rem[:, c, :], tag="lnsrc2"),
                       s_rem, slice(0, 512), ln_c["3s"], ln_c["3b"], p2)
            gemm_feat(dp["wqkv"], D, C8, [(lambda c: s_rem[:, c, :], 1)], p2,
                      ev_kT2)
            build_v_group(p2, wv2, s_rem, range(4, 8), v2_d, 4)

            oT2 = p2.tile([128, C8, T], F32R, name="oT2")
            attention(p2, qT2, kT2_d, v2_d, oT2)

            x3 = outer.tile([128, C8, T], F32R, tag="res512", bufs=2, name="x3")

            def ev_x3(nt, key, ps):
                x2c = p2.tile([128, 512], F32R, tag="lnsrc", bufs=2, name="x2c")
                nc.sync.dma_start(out=x2c[:], in_=x2_d[:, nt, :])
                nc.vector.scalar_tensor_tensor(
                    out=x3[:, nt, :], in0=ps[:], scalar=bso_c[:, nt:nt + 1],
                    in1=x2c[:], op0=OP.add, op1=OP.add)

            gemm_feat(dp["wso"], 0, C8, [(lambda c: oT2[:, c, :], 0)], p2, ev_x3)

        # ================= Phase 3: MLP =================
        with tc.tile_pool(name="p3", bufs=1) as p3:
            m_in = p3.tile([128, C8, T], F32R, name="m_in")
            layer_norm(lambda c: x3[:, c, :], m_in, slice(0, 512),
                       ln_c["4s"], ln_c["4b"], p3)

            hT = p3.tile([128, 32, T], F32R, name="hT")

            def ev_h(ht, key, ps):
                nc.scalar.activation(out=hT[:, ht, :], in_=ps[:],
                                     func=AF.Gelu_apprx_tanh,
                                     bias=b1_c[:, ht:ht + 1], scale=1.0)

            gemm_feat(dp["w1"], 0, 32, [(lambda c: m_in[:, c, :], 0)], p3, ev_h)

            w2r = _rearr_w(dp["w2"])  # [128, 32, D]
            outT_r = dp["outT"].rearrange("(c p) t -> p c t", p=128)
            for nt in range(C8):
                w2t = p3.tile([128, 32, 128], F32R, tag="w2t", bufs=2,
                              name="w2t")
                nc.sync.dma_start(out=w2t[:],
                                  in_=w2r[:, :, nt * 128:(nt + 1) * 128])
                ps = pp.tile([128, 512], F32, tag="mm", bufs=3, name="ops2")
                for kk in range(32):
                    nc.tensor.matmul(ps[:], w2t[:, kk, :], hT[:, kk, :],
                                     start=(kk == 0), stop=(kk == 31))
                ot = p3.tile([128, 512], F32R, tag="spill", bufs=2, name="ot")
                nc.vector.tensor_scalar_add(ot[:], ps[:],
                                            scalar1=b2_c[:, nt:nt + 1])
                nc.sync.dma_start(out=outT_r[:, nt, :], in_=ot[:])


def _get_program():
    if "nc" not in _PROGRAM_CACHE:
        _PROGRAM_CACHE["nc"] = _build_program()
    return _PROGRAM_CACHE["nc"]


def kernel(**inputs) -> np.ndarray:
    from concourse.bass_utils import run_bass_kernel_spmd

    nc = _get_program()

    x = np.asarray(inputs["x"], np.float32)
    key_val = np.asarray(inputs["key_val"], np.float32)
    f32 = lambda a: np.ascontiguousarray(np.asarray(a, np.float32))
    shared = {
        "wq": f32(inputs["Wq"]), "wkv": f32(inputs["Wkv"]),
        "wqkv": f32(inputs["Wqkv"]), "wco": f32(inputs["Wco"]),
        "wso": f32(inputs["Wso"]), "w1": f32(inputs["W1"]),
        "w2": f32(inputs["W2"]),
        "ln1_s": f32(inputs["ln1_s"])[None, :], "ln1_b": f32(inputs["ln1_b"])[None, :],
        "ln2_s": f32(inputs["ln2_s"])[None, :], "ln2_b": f32(inputs["ln2_b"])[None, :],
        "ln3_s": f32(inputs["ln3_s"])[None, :], "ln3_b": f32(inputs["ln3_b"])[None, :],
        "ln4_s": f32(inputs["ln4_s"])[None, :], "ln4_b": f32(inputs["ln4_b"])[None, :],
        "bco": f32(inputs["bco"])[None, :], "bso": f32(inputs["bso"])[None, :],
        "b1": f32(inputs["b1"])[None, :], "b2": f32(inputs["b2"])[None, :],
        "ones": np.ones((128, 128), np.float32),
    }
    in_maps = []
    for c in range(N_CORES):
        b, s = c // 2, c % 2
        m = dict(shared)
        m["xT"] = np.ascontiguousarray(x[b, s * T:(s + 1) * T, :].T)
        m["kvT"] = np.ascontiguousarray(key_val[b].T)
        in_maps.append(m)

    res = run_bass_kernel_spmd(nc, in_maps, list(range(N_CORES)))
    _PROGRAM_CACHE["last_result"] = res

    out = np.empty((B, NSEQ, D), np.float32)
    for c in range(N_CORES):
        b, s = c // 2, c % 2
        out[b, s * T:(s + 1) * T, :] = res.results[c]["outT"].T
    return out



# revision 19
# speedup vs baseline: 2.0628x; 2.0628x over previous
"""CrossAttnBlock kernel for 8 Trainium2 NeuronCores.

Sharding: core c -> (batch b = c//2, token-half s = c%2), 512 query tokens
per core. Cross-attention K/V is computed fully per core (duplicated within
the pair); after cross-attention the per-core residual x2 (bf16) is
exchanged with a pair-local 2-rank AllGather so each core rebuilds the
partner half's self-attn K/V locally (attention is permutation-invariant
over KV tokens, so own tokens always sit at positions 0:512).

All matmul operands are bf16 (fp32 PSUM accumulation); the residual stream
stays fp32. Weights are pre-shuffled on the host into [128, chunk*cin*ncol]
layout so every weight DMA is a single 2 MB transfer with 16 KB contiguous
per-partition lines. LayerNorm scale/bias are folded into the weights on the
host; the device computes only (x - mean) * rsqrt(var + eps), with
rsqrt obtained as exp(-0.5*ln(var+eps)) to stay inside one ACT table set.
K/V live entirely in SBUF. Softmax denominators come from a ones-column in
V; normalization uses reciprocal_approx_fast on all 16 heads at once plus a
selector-matrix matmul that broadcasts 1/Z to all 128 partitions per chunk.
"""
import sys

sys.path.insert(0, '/opt/trn_rl_repo')

import numpy as np
import concourse.bass as bass
from concourse import bacc
import concourse.tile as tile
from concourse import mybir

F32R = mybir.dt.float32r
F32 = mybir.dt.float32
BF = mybir.dt.bfloat16
AF = mybir.ActivationFunctionType
OP = mybir.AluOpType

N_CORES = 8
B, NSEQ, D, H, HD = 4, 1024, 1024, 16, 64
T = 512            # tokens owned per core
TF = 1024          # full token count per batch
C8 = D // 128      # feature chunks
CHUNK = 8 * 1024   # weight-chunk elems per partition (2 MB bf16 per chunk)
SCALE = 1.0 / float(np.sqrt(np.float32(HD)))
EPS = 1e-6

_PROGRAM_CACHE = {}

# weight name -> (n_chunks, cin_per_chunk, ncol_per_chunk)
W_SPECS = {
    "wq": (1, 8, 1024),
    "wkv": (2, 8, 1024),
    "wco": (1, 8, 1024),
    "wqkv": (3, 8, 1024),
    "wso": (1, 8, 1024),
    "w1": (4, 8, 1024),
    "w2": (4, 32, 256),
}


def _build_program():
    nc = bacc.Bacc("TRN2", target_bir_lowering=False, debug=False,
                   num_devices=N_CORES)

    dp = {}
    dp["xT"] = nc.declare_dram_parameter("xT", [128, C8 * T], F32R,
                                         isOutput=False)
    dp["kvT"] = nc.declare_dram_parameter("kvT", [128, C8 * TF], BF,
                                          isOutput=False)
    for nm, (q, _, _) in W_SPECS.items():
        dp[nm] = nc.declare_dram_parameter(nm, [128, q * CHUNK], BF,
                                           isOutput=False)
    # feature-major (column) biases
    for nm, n in [("bq", D), ("bk", D), ("bqs", D), ("bks", D),
                  ("bco", D), ("bso", D), ("b1", 4 * D), ("b2", D)]:
        dp[nm] = nc.declare_dram_parameter(nm, [1, n], F32, isOutput=False)
    # token-major (row) biases for the V projections
    for nm in ("bv", "bvs"):
        dp[nm] = nc.declare_dram_parameter(nm, [1, D], F32R, isOutput=False)
    dp["ones"] = nc.declare_dram_parameter("ones", [128, 128], F32R,
                                           isOutput=False)
    dp["ones_bf"] = nc.declare_dram_parameter("ones_bf", [128, 128], BF,
                                              isOutput=False)
    dp["esel"] = nc.declare_dram_parameter("esel", [16, C8 * 128], BF,
                                           isOutput=False)
    dp["eye16"] = nc.declare_dram_parameter("eye16", [1, 256], F32R,
                                            isOutput=False)
    dp["outT"] = nc.declare_dram_parameter("outT", [128, C8 * T], F32,
                                           isOutput=True)

    with tile.TileContext(nc) as tc:
        _emit(nc, tc, dp)
    nc.compile()
    return nc


def _emit(nc, tc, dp):
    import contextlib

    ctx = contextlib.ExitStack()
    with ctx:
        consts = ctx.enter_context(tc.tile_pool(name="consts", bufs=1))
        mp = ctx.enter_context(tc.tile_pool(name="main", bufs=1))
        pp = ctx.enter_context(tc.tile_pool(name="pp", bufs=1, space="PSUM"))
        small = ctx.enter_context(tc.tile_pool(name="small", bufs=1))
        dramp = ctx.enter_context(tc.tile_pool(name="dramp", bufs=1,
                                               space="DRAM"))

        # ---------- constants ----------
        ones_sb = consts.tile([128, 128], F32R)
        nc.sync.dma_start(out=ones_sb[:], in_=dp["ones"][:])
        ones_col = ones_sb[:, 0:1]
        ones_row = ones_sb[0:1, :]
        ones_bf = consts.tile([128, 128], BF)
        nc.sync.dma_start(out=ones_bf[:], in_=dp["ones_bf"][:])
        esel = consts.tile([16, C8, 128], BF)
        nc.sync.dma_start(out=esel[:],
                          in_=dp["esel"].rearrange("p (c n) -> p c n", c=C8))
        eye16 = consts.tile([1, 256], F32R)
        nc.sync.dma_start(out=eye16[:], in_=dp["eye16"][:])
        eps_t = consts.tile([1, 1], F32)
        nc.vector.memset(eps_t[:], EPS)

        def load_col(name, nchunk):
            col = consts.tile([128, nchunk], F32, name=f"col_{name}")
            nc.sync.dma_start(
                out=col[:], in_=dp[name].rearrange("o (c p) -> p (o c)", p=128))
            return col

        bq_c = load_col("bq", C8)
        bk_c = load_col("bk", C8)
        bqs_c = load_col("bqs", C8)
        bks_c = load_col("bks", C8)
        bco_c = load_col("bco", C8)
        bso_c = load_col("bso", C8)
        b1_c = load_col("b1", 32)
        b2_c = load_col("b2", C8)
        bv_r = consts.tile([1, D], F32R, name="bv_r")
        nc.sync.dma_start(out=bv_r[:], in_=dp["bv"][:])
        bvs_r = consts.tile([1, D], F32R, name="bvs_r")
        nc.sync.dma_start(out=bvs_r[:], in_=dp["bvs"][:])

        pid = nc.sync.partition_id()
        rem_idx = 1 - pid % 2

        # ---------- DRAM intermediates ----------
        x2bf_d = dramp.tile([128, C8 * T], BF, name="x2bf_d")
        ag_out = dramp.tile([2, 128, C8 * T], BF, name="ag_out")

        # ---------- helpers ----------
        def load_wchunk(name, q, cch, nch):
            t = mp.tile([128, cch, nch], BF, tag="wc", bufs=2,
                        name=f"w_{name}{q}")
            nc.sync.dma_start(
                out=t[:],
                in_=dp[name][:, q * CHUNK:(q + 1) * CHUNK].rearrange(
                    "p (c n) -> p c n", c=cch))
            return t

        def layer_norm(src_fn, src_dt, dst, dst_sl):
            """dst[:, c, dst_sl] = (src - mean) * rsqrt(var + eps), bf16 out.

            src_fn(c) -> [128, 512] AP of dtype src_dt (F32R or BF).
            """
            ones_c = ones_col if src_dt == F32R else ones_bf[:, 0:1]
            stats_x = pp.tile([1, 512], F32, tag="mm", bufs=2, name="stats_x")
            stats_q = pp.tile([1, 512], F32, tag="mm", bufs=2, name="stats_q")
            for c in range(C8):
                xc = src_fn(c)
                sq = mp.tile([128, 512], F32R, tag="sq", bufs=2, name="sq")
                nc.scalar.activation(out=sq[:], in_=xc, func=AF.Square)
                nc.tensor.matmul(stats_x[:], ones_c, xc,
                                 start=(c == 0), stop=(c == C8 - 1),
                                 skip_group_check=True)
                nc.tensor.matmul(stats_q[:], ones_col, sq[:],
                                 start=(c == 0), stop=(c == C8 - 1),
                                 skip_group_check=True)
            mean = small.tile([1, 512], F32R, tag="mean", bufs=1, name="mean")
            nc.vector.tensor_scalar_mul(mean[:], stats_x[:], 1.0 / D)
            var = small.tile([1, 512], F32R, tag="var", bufs=1, name="var")
            nc.vector.tensor_scalar(var[:], stats_q[:],
                                    scalar1=1.0 / D, scalar2=0.0,
                                    op0=OP.mult, op1=OP.add)
            m2 = small.tile([1, 512], F32R, tag="lntmp", bufs=1, name="m2")
            nc.vector.tensor_mul(m2[:], mean[:], mean[:])
            nc.vector.tensor_sub(var[:], var[:], m2[:])
            lnv = small.tile([1, 512], F32R, tag="lntmp", bufs=1, name="lnv")
            nc.scalar.activation(out=lnv[:], in_=var[:], func=AF.Ln,
                                 bias=eps_t[:])
            u = small.tile([1, 512], F32R, tag="u", bufs=1, name="u")
            nc.scalar.activation(out=u[:], in_=lnv[:], func=AF.Exp,
                                 scale=-0.5)
            v = small.tile([1, 512], F32R, tag="lntmp", bufs=1, name="v")
            nc.vector.tensor_mul(v[:], mean[:], u[:])
            a0 = pp.tile([128, 1024], F32, tag="sc", bufs=2, name="a0")
            nc.tensor.matmul(a0[:, 0:512], ones_row, u[:], start=True,
                             stop=True)
            c0 = pp.tile([128, 1024], F32, tag="sc", bufs=2, name="c0")
            nc.tensor.matmul(c0[:, 0:512], ones_row, v[:], start=True,
                             stop=True)
            if src_dt == F32R:
                a0r, c0r = a0[:, 0:512], c0[:, 0:512]
            else:
                a0s = mp.tile([128, 512], BF, tag="a0s", bufs=2, name="a0s")
                nc.scalar.activation(out=a0s[:], in_=a0[:, 0:512],
                                     func=AF.Copy)
                c0s = mp.tile([128, 512], BF, tag="a0s", bufs=2, name="c0s")
                nc.scalar.activation(out=c0s[:], in_=c0[:, 0:512],
                                     func=AF.Copy)
                a0r, c0r = a0s[:], c0s[:]
            for c in range(C8):
                xc = src_fn(c)
                tmp = mp.tile([128, 512], F32R, tag="lnt", bufs=2, name="lnt")
                nc.vector.tensor_mul(tmp[:], xc, a0r)
                nc.vector.tensor_sub(dst[:, c, dst_sl], tmp[:], c0r)

        def gemm(wt, rhs_fn, n_tiles, evict, nt_off=0):
            """Standard feature-major GEMM from one weight chunk.

            wt: [128, 8, 1024] bf16 chunk; rhs_fn(c) -> [128, 512] bf16 AP.
            evict(nt, ps) with nt in [nt_off, nt_off + n_tiles).
            """
            for nt in range(n_tiles):
                ps = pp.tile([128, 512], F32, tag="mm", bufs=2, name="gps")
                for c in range(C8):
                    nc.tensor.matmul(ps[:], wt[:, c, nt * 128:(nt + 1) * 128],
                                     rhs_fn(c), start=(c == 0),
                                     stop=(c == C8 - 1))
                evict(nt_off + nt, ps)

        def build_v(wt, src, jts, v_dst, jt_off, bias_row):
            """Token-major V tiles for j-tiles jts, written into v_dst.

            wt: V weight chunk [128, 8, 1024]; src: [128, C8, 512] bf16;
            v_dst: [128, 8, 16, 65] bf16. jts index into v_dst; jt_off maps
            them onto src token positions.
            """
            for dvh in range(2):
                for jt in jts:
                    ps = pp.tile([128, 512], F32, tag="mm", bufs=2, name="vps")
                    sl = slice((jt - jt_off) * 128, (jt - jt_off + 1) * 128)
                    for c in range(C8):
                        nc.tensor.matmul(ps[:], src[:, c, sl],
                                         wt[:, c, dvh * 512:(dvh + 1) * 512],
                                         start=(c == 0), stop=False)
                    nc.tensor.matmul(ps[:], ones_row[0:1, 0:128],
                                     bias_row[0:1, dvh * 512:(dvh + 1) * 512],
                                     start=False, stop=True)
                    nc.vector.tensor_copy(
                        out=v_dst[:, jt, dvh * 8:(dvh + 1) * 8, 0:64],
                        in_=ps[:].rearrange("p (h e) -> p h e", h=8))

        def attention(qT, kT, vv, oT):
            """oT (bf16, feature-major, unnormalized->normalized in place)."""
            zall_ps = pp.tile([16, 512], F32, tag="mm", bufs=2, name="zall_ps")
            for g in range(C8):  # head pair (2g, 2g+1)
                o_ps = []
                for hh in range(2):
                    o_ps.append(pp.tile([65, 512], F32, tag="ops", bufs=2,
                                        name="ops"))
                for jt2 in range(4):
                    pts = []
                    for hh in range(2):
                        off = hh * 64
                        s_ps = pp.tile([128, 1024], F32, tag="sc", bufs=2,
                                       name="sps")
                        for j in range(2):
                            jt = 2 * jt2 + j
                            nc.tensor.matmul(
                                s_ps[:, j * 512:(j + 1) * 512],
                                kT[off:off + 64, g, jt * 128:(jt + 1) * 128],
                                qT[off:off + 64, g, :],
                                start=True, stop=True)
                        pt = mp.tile([128, 1024], BF, tag="pt", bufs=3,
                                     name="pt")
                        nc.scalar.activation(out=pt[:], in_=s_ps[:],
                                             func=AF.Exp, scale=SCALE)
                        pts.append(pt)
                    for hh in range(2):
                        h = 2 * g + hh
                        for j in range(2):
                            jt = 2 * jt2 + j
                            nc.tensor.matmul(
                                o_ps[hh][:], vv[:, jt, h, :],
                                pts[hh][:, j * 512:(j + 1) * 512],
                                start=(jt == 0), stop=(jt == 7),
                                skip_group_check=True)
                for hh in range(2):
                    h = 2 * g + hh
                    off = hh * 64
                    nc.vector.tensor_copy(out=oT[off:off + 64, g, :],
                                          in_=o_ps[hh][0:64, :])
                    zrow = small.tile([1, 512], F32R, tag="zrow", bufs=1,
                                      name="zrow")
                    nc.vector.tensor_copy(out=zrow[:], in_=o_ps[hh][64:65, :])
                    # scatter Z_h into partition h of zall_ps via a K=1 matmul
                    nc.tensor.matmul(zall_ps[:], eye16[0:1, h * 16:h * 16 + 16],
                                     zrow[:], start=(h == 0), stop=(h == 15),
                                     skip_group_check=True)
            zall = small.tile([16, 512], F32, tag="zall", bufs=1, name="zall")
            nc.vector.tensor_copy(out=zall[:], in_=zall_ps[:])
            nc.vector.reciprocal_approx_fast(out=zall[:], in_=zall[:])
            zinv_bf = small.tile([16, 512], BF, tag="zinvb", bufs=1,
                                 name="zinvb")
            nc.vector.tensor_copy(out=zinv_bf[:], in_=zall[:])
            for g in range(C8):
                zb = pp.tile([128, 512], F32, tag="mm", bufs=2, name="zb")
                nc.tensor.matmul(zb[:], esel[:, g, :], zinv_bf[:],
                                 start=True, stop=True)
                nc.vector.tensor_mul(oT[:, g, :], oT[:, g, :], zb[:])

        # ---------- load inputs ----------
        x1 = mp.tile([128, C8, T], F32R, tag="res", bufs=2, name="x1")
        nc.sync.dma_start(out=x1[:],
                          in_=dp["xT"].rearrange("p (c t) -> p c t", c=C8))
        kvT_r = dp["kvT"].rearrange("p (c t) -> p c t", c=C8)

        def kv_src(c, sl):
            t = mp.tile([128, 512], BF, tag="lnsrc", bufs=2, name="kvs")
            nc.sync.dma_start(out=t[:], in_=kvT_r[:, c, sl])
            return t[:]

        # ================= Phase 1: cross-attention =================
        q_in = mp.tile([128, C8, T], BF, tag="lnout", bufs=2, name="q_in")
        layer_norm(lambda c: x1[:, c, :], F32R, q_in, slice(0, T))

        qT1 = mp.tile([128, C8, T], BF, tag="qT", bufs=1, name="qT1")

        def ev_qT1(nt, ps):
            nc.vector.tensor_scalar_add(qT1[:, nt, :], ps[:],
                                        scalar1=bq_c[:, nt:nt + 1])

        wq_t = load_wchunk("wq", 0, 8, 1024)
        gemm(wq_t, lambda c: q_in[:, c, :], C8, ev_qT1)

        kT1 = mp.tile([128, C8, TF], BF, tag="kT", bufs=1, name="kT1")
        v1 = mp.tile([128, 8, H, 65], BF, tag="vv", bufs=1, name="v1")
        nc.vector.memset(v1[:, :, :, 64], 1.0)

        wk_t = load_wchunk("wkv", 0, 8, 1024)
        wv_t = load_wchunk("wkv", 1, 8, 1024)
        for th in range(2):
            sl = slice(th * T, (th + 1) * T)
            kv_in = mp.tile([128, C8, T], BF, tag="lnout", bufs=2,
                            name=f"kv_in{th}")
            layer_norm(lambda c, sl=sl: kv_src(c, sl), BF, kv_in, slice(0, T))

            def ev_kT1(nt, ps, sl=sl):
                nc.vector.tensor_scalar_add(kT1[:, nt, sl], ps[:],
                                            scalar1=bk_c[:, nt:nt + 1])

            gemm(wk_t, lambda c: kv_in[:, c, :], C8, ev_kT1)
            build_v(wv_t, kv_in, range(th * 4, th * 4 + 4), v1,
                    th * 4, bv_r)

        oT1 = mp.tile([128, C8, T], BF, tag="oT", bufs=1, name="oT1")
        attention(qT1, kT1, v1, oT1)

        # x2 = x1 + Wco @ o + bco ; also exported as bf16 for the pair AG
        x2 = mp.tile([128, C8, T], F32R, tag="res", bufs=2, name="x2")

        def ev_x2(nt, ps):
            nc.vector.scalar_tensor_tensor(
                out=x2[:, nt, :], in0=ps[:], scalar=bco_c[:, nt:nt + 1],
                in1=x1[:, nt, :], op0=OP.add, op1=OP.add)

        wco_t = load_wchunk("wco", 0, 8, 1024)
        gemm(wco_t, lambda c: oT1[:, c, :], C8, ev_x2)
        # casting DMA (SWDGE): f32 residual -> bf16 AllGather payload
        nc.gpsimd.dma_start(out=x2bf_d[:],
                            in_=x2[:].rearrange("p c t -> p (c t)"))

        # ================= x2 exchange (pair AllGather) =================
        nc.gpsimd.collective_compute(
            "AllGather", OP.bypass,
            ins=[x2bf_d[:]],
            outs=[ag_out[:]],
            replica_groups=[[0, 1], [2, 3], [4, 5], [6, 7]])

        # ================= Phase 2: self-attention =================
        s_own = mp.tile([128, C8, T], BF, tag="lnout", bufs=2, name="s_own")
        layer_norm(lambda c: x2[:, c, :], F32R, s_own, slice(0, T))

        qT2 = mp.tile([128, C8, T], BF, tag="qT", bufs=1, name="qT2")
        kT2 = mp.tile([128, C8, TF], BF, tag="kT", bufs=1, name="kT2")
        v2 = mp.tile([128, 8, H, 65], BF, tag="vv", bufs=1, name="v2")
        nc.vector.memset(v2[:, :, :, 64], 1.0)

        def ev_qT2(nt, ps):
            nc.vector.tensor_scalar_add(qT2[:, nt, :], ps[:],
                                        scalar1=bqs_c[:, nt:nt + 1])

        wqs_t = load_wchunk("wqkv", 0, 8, 1024)
        gemm(wqs_t, lambda c: s_own[:, c, :], C8, ev_qT2)

        def ev_kT2_own(nt, ps):
            nc.vector.tensor_scalar_add(kT2[:, nt, 0:T], ps[:],
                                        scalar1=bks_c[:, nt:nt + 1])

        wks_t = load_wchunk("wqkv", 1, 8, 1024)
        gemm(wks_t, lambda c: s_own[:, c, :], C8, ev_kT2_own)
        wvs_t = load_wchunk("wqkv", 2, 8, 1024)
        build_v(wvs_t, s_own, range(0, 4), v2, 0, bvs_r)

        # remote half (depends on the AllGather)
        ag_rem = ag_out[bass.ds(rem_idx, 1), :, :].rearrange(
            "o p (c t) -> p (o c) t", c=C8)
        s_rem = mp.tile([128, C8, T], BF, tag="lnout", bufs=2, name="s_rem")

        def rem_src(c):
            t = mp.tile([128, 512], BF, tag="lnsrc", bufs=2, name="agrem")
            nc.sync.dma_start(out=t[:], in_=ag_rem[:, c, :])
            return t[:]

        layer_norm(rem_src, BF, s_rem, slice(0, T))

        def ev_kT2_rem(nt, ps):
            nc.vector.tensor_scalar_add(kT2[:, nt, T:TF], ps[:],
                                        scalar1=bks_c[:, nt:nt + 1])

        gemm(wks_t, lambda c: s_rem[:, c, :], C8, ev_kT2_rem)
        build_v(wvs_t, s_rem, range(4, 8), v2, 4, bvs_r)

        oT2 = mp.tile([128, C8, T], BF, tag="oT", bufs=1, name="oT2")
        attention(qT2, kT2, v2, oT2)

        x3 = mp.tile([128, C8, T], F32R, tag="res", bufs=2, name="x3")

        def ev_x3(nt, ps):
            nc.vector.scalar_tensor_tensor(
                out=x3[:, nt, :], in0=ps[:], scalar=bso_c[:, nt:nt + 1],
                in1=x2[:, nt, :], op0=OP.add, op1=OP.add)

        wso_t = load_wchunk("wso", 0, 8, 1024)
        gemm(wso_t, lambda c: oT2[:, c, :], C8, ev_x3)

        # ================= Phase 3: MLP =================
        m_in = mp.tile([128, C8, T], BF, tag="lnout", bufs=2, name="m_in")
        layer_norm(lambda c: x3[:, c, :], F32R, m_in, slice(0, T))

        hT = mp.tile([128, 32, T], BF, tag="hT", bufs=1, name="hT")

        def ev_h(ht, ps):
            nc.scalar.activation(out=hT[:, ht, :], in_=ps[:],
                                 func=AF.Gelu_apprx_tanh,
                                 bias=b1_c[:, ht:ht + 1], scale=1.0)

        for q in range(4):
            w1_t = load_wchunk("w1", q, 8, 1024)
            gemm(w1_t, lambda c: m_in[:, c, :], C8, ev_h, nt_off=q * C8)

        outT_r = dp["outT"].rearrange("p (c t) -> p c t", c=C8)
        for q in range(4):
            w2_t = load_wchunk("w2", q, 32, 256)
            for nt2 in range(2):
                nt = q * 2 + nt2
                ps = pp.tile([128, 512], F32, tag="mm", bufs=2, name="ops2")
                for kk in range(32):
                    nc.tensor.matmul(
                        ps[:], w2_t[:, kk, nt2 * 128:(nt2 + 1) * 128],
                        hT[:, kk, :], start=(kk == 0), stop=(kk == 31))
                ot = mp.tile([128, 512], F32, tag="otile", bufs=2, name="ot")
                nc.vector.tensor_scalar_add(ot[:], ps[:],
                                            scalar1=b2_c[:, nt:nt + 1])
                nc.sync.dma_start(out=outT_r[:, nt, :], in_=ot[:])


def _get_program():
    if "nc" not in _PROGRAM_CACHE:
        _PROGRAM_CACHE["nc"] = _build_program()
    return _PROGRAM_CACHE["nc"]


def _pack_weight(w, npc):
    """[Din, Dout] f32 -> [128, Q*CHUNK] bf16 with per-partition-contiguous
    2MB chunks of layout [chunk][cin][ncol]."""
    import ml_dtypes
    din, dout = w.shape
    c = din // 128
    q = dout // npc
    arr = np.ascontiguousarray(w).astype(ml_dtypes.bfloat16)
    arr = arr.reshape(c, 128, q, npc).transpose(1, 2, 0, 3)
    return np.ascontiguousarray(arr.reshape(128, q * c * npc))


def kernel(**inputs) -> np.ndarray:
    import ml_dtypes
    from concourse.bass_utils import run_bass_kernel_spmd

    nc = _get_program()

    f32 = lambda a: np.ascontiguousarray(np.asarray(a, np.float32))
    x = f32(inputs["x"])
    key_val = f32(inputs["key_val"])
    s1, b1n = f32(inputs["ln1_s"]), f32(inputs["ln1_b"])
    s2, b2n = f32(inputs["ln2_s"]), f32(inputs["ln2_b"])
    s3, b3n = f32(inputs["ln3_s"]), f32(inputs["ln3_b"])
    s4, b4n = f32(inputs["ln4_s"]), f32(inputs["ln4_b"])
    wq = f32(inputs["Wq"]) * s1[:, None]
    wkv = f32(inputs["Wkv"]) * s2[:, None]
    wqkv = f32(inputs["Wqkv"]) * s3[:, None]
    w1 = f32(inputs["W1"]) * s4[:, None]
    wco, wso, w2 = f32(inputs["Wco"]), f32(inputs["Wso"]), f32(inputs["W2"])
    bq = f32(inputs["Wq"]).T @ b1n
    bkv = f32(inputs["Wkv"]).T @ b2n
    bqkv = f32(inputs["Wqkv"]).T @ b3n
    b1f = f32(inputs["b1"]) + f32(inputs["W1"]).T @ b4n

    esel = np.zeros((16, C8 * 128), np.float32)
    for g in range(C8):
        esel[2 * g, g * 128:g * 128 + 64] = 1.0
        esel[2 * g + 1, g * 128 + 64:(g + 1) * 128] = 1.0

    bf = lambda a: np.ascontiguousarray(np.asarray(a, ml_dtypes.bfloat16))
    feat_major = lambda a, dt: np.ascontiguousarray(
        a.T.reshape(C8, 128, -1).transpose(1, 0, 2).reshape(128, -1)
        .astype(dt))

    shared = {
        "wq": _pack_weight(wq, 1024), "wkv": _pack_weight(wkv, 1024),
        "wqkv": _pack_weight(wqkv, 1024), "wco": _pack_weight(wco, 1024),
        "wso": _pack_weight(wso, 1024), "w1": _pack_weight(w1, 1024),
        "w2": _pack_weight(w2, 256),
        "bq": f32(bq)[None, :], "bk": f32(bkv[:D])[None, :],
        "bv": f32(bkv[D:])[None, :],
        "bqs": f32(bqkv[:D])[None, :], "bks": f32(bqkv[D:2 * D])[None, :],
        "bvs": f32(bqkv[2 * D:])[None, :],
        "bco": f32(inputs["bco"])[None, :], "bso": f32(inputs["bso"])[None, :],
        "b1": f32(b1f)[None, :], "b2": f32(inputs["b2"])[None, :],
        "ones": np.ones((128, 128), np.float32),
        "ones_bf": bf(np.ones((128, 128))),
        "esel": bf(esel),
        "eye16": np.eye(16, dtype=np.float32).reshape(1, 256),
    }
    in_maps = []
    for c in range(N_CORES):
        b, s = c // 2, c % 2
        m = dict(shared)
        m["xT"] = feat_major(x[b, s * T:(s + 1) * T, :], np.float32)
        m["kvT"] = feat_major(key_val[b], ml_dtypes.bfloat16)
        in_maps.append(m)

    res = run_bass_kernel_spmd(nc, in_maps, list(range(N_CORES)))
    _PROGRAM_CACHE["last_result"] = res

    out = np.empty((B, NSEQ, D), np.float32)
    for c in range(N_CORES):
        b, s = c // 2, c % 2
        r = np.asarray(res.results[c]["outT"], np.float32)
        r = r.reshape(128, C8, T).transpose(1, 0, 2).reshape(D, T)
        out[b, s * T:(s + 1) * T, :] = r.T
    return out


# revision 22
# speedup vs baseline: 2.1664x; 1.0502x over previous
"""CrossAttnBlock kernel for 8 Trainium2 NeuronCores.

Sharding: core c -> (batch b = c//2, token-half s = c%2), 512 query tokens
per core. Cross-attention K/V is computed fully per core (duplicated within
the pair); after cross-attention the per-core residual x2 (bf16) is
exchanged with a pair-local 2-rank AllGather so each core rebuilds the
partner half's self-attn K/V locally (attention is permutation-invariant
over KV tokens, so own tokens always sit at positions 0:512).

All matmul operands are bf16 (fp32 PSUM accumulation); the residual stream
stays fp32. Weights are pre-shuffled on the host into [128, chunk*cin*ncol]
layout so every weight DMA is a single 2 MB transfer with 16 KB contiguous
per-partition lines. LayerNorm scale/bias are folded into the weights on the
host; the device computes only (x - mean) * rsqrt(var + eps), with
rsqrt obtained as exp(-0.5*ln(var+eps)) to stay inside one ACT table set.
K/V live entirely in SBUF. Softmax denominators come from a ones-column in
V; normalization uses reciprocal_approx_fast on all 16 heads at once plus a
selector-matrix matmul that broadcasts 1/Z to all 128 partitions per chunk.
"""
import sys

sys.path.insert(0, '/opt/trn_rl_repo')

import numpy as np
import concourse.bass as bass
from concourse import bacc
import concourse.tile as tile
from concourse import mybir

F32R = mybir.dt.float32r
F32 = mybir.dt.float32
BF = mybir.dt.bfloat16
AF = mybir.ActivationFunctionType
OP = mybir.AluOpType

N_CORES = 8
B, NSEQ, D, H, HD = 4, 1024, 1024, 16, 64
T = 512            # tokens owned per core
TF = 1024          # full token count per batch
C8 = D // 128      # feature chunks
CHUNK = 8 * 1024   # weight-chunk elems per partition (2 MB bf16 per chunk)
SCALE = 1.0 / float(np.sqrt(np.float32(HD)))
EPS = 1e-6

_PROGRAM_CACHE = {}

# weight name -> (n_chunks, cin_per_chunk, ncol_per_chunk)
W_SPECS = {
    "wq": (1, 8, 1024),
    "wkv": (2, 8, 1024),
    "wco": (1, 8, 1024),
    "wqkv": (3, 8, 1024),
    "wso": (1, 8, 1024),
    "w1": (4, 8, 1024),
    "w2": (4, 32, 256),
}


def _build_program():
    nc = bacc.Bacc("TRN2", target_bir_lowering=False, debug=False,
                   num_devices=N_CORES)

    dp = {}
    dp["xT"] = nc.declare_dram_parameter("xT", [128, C8 * T], F32R,
                                         isOutput=False)
    dp["kvT"] = nc.declare_dram_parameter("kvT", [128, C8 * TF], BF,
                                          isOutput=False)
    for nm, (q, _, _) in W_SPECS.items():
        dp[nm] = nc.declare_dram_parameter(nm, [128, q * CHUNK], BF,
                                           isOutput=False)
    # feature-major (column) biases
    for nm, n in [("bq", D), ("bk", D), ("bqs", D), ("bks", D),
                  ("bco", D), ("bso", D), ("b1", 4 * D), ("b2", D)]:
        dp[nm] = nc.declare_dram_parameter(nm, [1, n], F32, isOutput=False)
    dp["ones"] = nc.declare_dram_parameter("ones", [128, 128], F32R,
                                           isOutput=False)
    dp["ones_bf"] = nc.declare_dram_parameter("ones_bf", [128, 128], BF,
                                              isOutput=False)
    dp["esel"] = nc.declare_dram_parameter("esel", [16, C8 * 128], BF,
                                           isOutput=False)
    dp["eye16"] = nc.declare_dram_parameter("eye16", [1, 256], F32R,
                                            isOutput=False)
    dp["outT"] = nc.declare_dram_parameter("outT", [128, C8 * T], F32,
                                           isOutput=True)

    with tile.TileContext(nc) as tc:
        _emit(nc, tc, dp)
    nc.compile()
    return nc


def _emit(nc, tc, dp):
    import contextlib

    ctx = contextlib.ExitStack()
    with ctx:
        consts = ctx.enter_context(tc.tile_pool(name="consts", bufs=1))
        mp = ctx.enter_context(tc.tile_pool(name="main", bufs=1))
        pp = ctx.enter_context(tc.tile_pool(name="pp", bufs=1, space="PSUM"))
        small = ctx.enter_context(tc.tile_pool(name="small", bufs=1))
        dramp = ctx.enter_context(tc.tile_pool(name="dramp", bufs=1,
                                               space="DRAM"))

        # ---------- constants ----------
        ones_sb = consts.tile([128, 128], F32R)
        nc.sync.dma_start(out=ones_sb[:], in_=dp["ones"][:])
        ones_col = ones_sb[:, 0:1]
        ones_row = ones_sb[0:1, :]
        ones_bf = consts.tile([128, 128], BF)
        nc.sync.dma_start(out=ones_bf[:], in_=dp["ones_bf"][:])
        esel = consts.tile([16, C8, 128], BF)
        nc.sync.dma_start(out=esel[:],
                          in_=dp["esel"].rearrange("p (c n) -> p c n", c=C8))
        eye16 = consts.tile([1, 256], F32R)
        nc.sync.dma_start(out=eye16[:], in_=dp["eye16"][:])
        eps_t = consts.tile([1, 1], F32)
        nc.vector.memset(eps_t[:], EPS)

        def load_col(name, nchunk):
            col = consts.tile([128, nchunk], F32, name=f"col_{name}")
            nc.sync.dma_start(
                out=col[:], in_=dp[name].rearrange("o (c p) -> p (o c)", p=128))
            return col

        bq_c = load_col("bq", C8)
        bk_c = load_col("bk", C8)
        bqs_c = load_col("bqs", C8)
        bks_c = load_col("bks", C8)
        bco_c = load_col("bco", C8)
        bso_c = load_col("bso", C8)
        b1_c = load_col("b1", 32)
        b2_c = load_col("b2", C8)

        pid = nc.sync.partition_id()
        rem_idx = 1 - pid % 2

        # ---------- DRAM intermediates ----------
        x2bf_d = dramp.tile([128, C8 * T], BF, name="x2bf_d")
        ag_out = dramp.tile([2, 128, C8 * T], BF, name="ag_out")

        # ---------- helpers ----------
        def load_wchunk(name, q, cch, nch):
            t = mp.tile([128, cch, nch], BF, tag="wc", bufs=2,
                        name=f"w_{name}{q}")
            nc.sync.dma_start(
                out=t[:],
                in_=dp[name][:, q * CHUNK:(q + 1) * CHUNK].rearrange(
                    "p (c n) -> p c n", c=cch))
            return t

        def layer_norm(src_fn, src_dt, dst, dst_sl, stag="mm"):
            """dst[:, c, dst_sl] = (src - mean) * rsqrt(var + eps), bf16 out.

            src_fn(c) -> [128, 512] AP of dtype src_dt (F32R or BF).
            """
            ones_c = ones_col if src_dt == F32R else ones_bf[:, 0:1]
            stats_x = pp.tile([1, 512], F32, tag=stag, bufs=2, name="stats_x")
            stats_q = pp.tile([1, 512], F32, tag=stag, bufs=2, name="stats_q")
            for c in range(C8):
                xc = src_fn(c)
                sq = mp.tile([128, 512], F32R, tag="sq", bufs=2, name="sq")
                nc.vector.tensor_mul(sq[:], xc, xc)
                nc.tensor.matmul(stats_x[:], ones_c, xc,
                                 start=(c == 0), stop=(c == C8 - 1),
                                 skip_group_check=True)
                nc.tensor.matmul(stats_q[:], ones_col, sq[:],
                                 start=(c == 0), stop=(c == C8 - 1),
                                 skip_group_check=True)
            mean = small.tile([1, 512], F32R, tag="mean", bufs=1, name="mean")
            nc.vector.tensor_scalar_mul(mean[:], stats_x[:], 1.0 / D)
            var = small.tile([1, 512], F32R, tag="var", bufs=1, name="var")
            nc.vector.tensor_scalar(var[:], stats_q[:],
                                    scalar1=1.0 / D, scalar2=0.0,
                                    op0=OP.mult, op1=OP.add)
            m2 = small.tile([1, 512], F32R, tag="lntmp", bufs=1, name="m2")
            nc.vector.tensor_mul(m2[:], mean[:], mean[:])
            nc.vector.tensor_sub(var[:], var[:], m2[:])
            lnv = small.tile([1, 512], F32R, tag="lntmp", bufs=1, name="lnv")
            nc.scalar.activation(out=lnv[:], in_=var[:], func=AF.Ln,
                                 bias=eps_t[:])
            u = small.tile([1, 512], F32R, tag="u", bufs=1, name="u")
            nc.scalar.activation(out=u[:], in_=lnv[:], func=AF.Exp,
                                 scale=-0.5)
            v = small.tile([1, 512], F32R, tag="lntmp", bufs=1, name="v")
            nc.vector.tensor_mul(v[:], mean[:], u[:])
            a0 = pp.tile([128, 1024], F32, tag="sc", bufs=2, name="a0")
            nc.tensor.matmul(a0[:, 0:512], ones_row, u[:], start=True,
                             stop=True)
            c0 = pp.tile([128, 1024], F32, tag="sc", bufs=2, name="c0")
            nc.tensor.matmul(c0[:, 0:512], ones_row, v[:], start=True,
                             stop=True)
            if src_dt == F32R:
                a0r, c0r = a0[:, 0:512], c0[:, 0:512]
            else:
                a0s = mp.tile([128, 512], BF, tag="a0s", bufs=2, name="a0s")
                nc.scalar.activation(out=a0s[:], in_=a0[:, 0:512],
                                     func=AF.Copy)
                c0s = mp.tile([128, 512], BF, tag="a0s", bufs=2, name="c0s")
                nc.scalar.activation(out=c0s[:], in_=c0[:, 0:512],
                                     func=AF.Copy)
                a0r, c0r = a0s[:], c0s[:]
            for c in range(C8):
                xc = src_fn(c)
                tmp = mp.tile([128, 512], F32R, tag="lnt", bufs=2, name="lnt")
                nc.vector.tensor_mul(tmp[:], xc, a0r)
                nc.vector.tensor_sub(dst[:, c, dst_sl], tmp[:], c0r)

        def gemm(wt, rhs_fn, n_tiles, evict, nt_off=0):
            """Standard feature-major GEMM from one weight chunk.

            wt: [128, 8, 1024] bf16 chunk; rhs_fn(c) -> [128, 512] bf16 AP.
            evict(nt, ps) with nt in [nt_off, nt_off + n_tiles).
            """
            for nt in range(n_tiles):
                ps = pp.tile([128, 512], F32, tag="mm", bufs=2, name="gps")
                for c in range(C8):
                    nc.tensor.matmul(ps[:], wt[:, c, nt * 128:(nt + 1) * 128],
                                     rhs_fn(c), start=(c == 0),
                                     stop=(c == C8 - 1))
                evict(nt_off + nt, ps)

        def build_v(wt, src, jts, v_dst, jt_off):
            """Token-major V tiles for j-tiles jts, written into v_dst.

            wt: V weight chunk [128, 8, 1024]; src: [128, C8, 512] bf16;
            v_dst: [128, 8, 16, 65] bf16. jts index into v_dst; jt_off maps
            them onto src token positions. (V bias is folded into bco/bso
            on the host: softmax rows sum to 1.)
            """
            for dvh in range(2):
                for jt in jts:
                    ps = pp.tile([128, 512], F32, tag="mm", bufs=2, name="vps")
                    sl = slice((jt - jt_off) * 128, (jt - jt_off + 1) * 128)
                    for c in range(C8):
                        nc.tensor.matmul(ps[:], src[:, c, sl],
                                         wt[:, c, dvh * 512:(dvh + 1) * 512],
                                         start=(c == 0), stop=(c == C8 - 1))
                    nc.vector.tensor_copy(
                        out=v_dst[:, jt, dvh * 8:(dvh + 1) * 8, 0:64],
                        in_=ps[:].rearrange("p (h e) -> p h e", h=8))

        def attention(qT, kT, vv, oT):
            """oT (bf16, feature-major, unnormalized->normalized in place)."""
            zall_ps = pp.tile([16, 512], F32, tag="mm", bufs=2, name="zall_ps")
            for g in range(C8):  # head pair (2g, 2g+1)
                o_ps = []
                for hh in range(2):
                    o_ps.append(pp.tile([65, 512], F32, tag="ops", bufs=2,
                                        name="ops"))
                for jt2 in range(4):
                    pts = []
                    for hh in range(2):
                        off = hh * 64
                        s_ps = pp.tile([128, 1024], F32, tag="sc", bufs=2,
                                       name="sps")
                        for j in range(2):
                            jt = 2 * jt2 + j
                            nc.tensor.matmul(
                                s_ps[:, j * 512:(j + 1) * 512],
                                kT[off:off + 64, g, jt * 128:(jt + 1) * 128],
                                qT[off:off + 64, g, :],
                                start=True, stop=True)
                        pt = mp.tile([128, 1024], BF, tag="pt", bufs=3,
                                     name="pt")
                        nc.scalar.activation(out=pt[:], in_=s_ps[:],
                                             func=AF.Exp, scale=SCALE)
                        pts.append(pt)
                    for hh in range(2):
                        h = 2 * g + hh
                        for j in range(2):
                            jt = 2 * jt2 + j
                            nc.tensor.matmul(
                                o_ps[hh][:], vv[:, jt, h, :],
                                pts[hh][:, j * 512:(j + 1) * 512],
                                start=(jt == 0), stop=(jt == 7),
                                skip_group_check=True)
                for hh in range(2):
                    h = 2 * g + hh
                    off = hh * 64
                    nc.vector.tensor_copy(out=oT[off:off + 64, g, :],
                                          in_=o_ps[hh][0:64, :])
                    zrow = small.tile([1, 512], F32R, tag="zrow", bufs=1,
                                      name="zrow")
                    nc.vector.tensor_copy(out=zrow[:], in_=o_ps[hh][64:65, :])
                    # scatter Z_h into partition h of zall_ps via a K=1 matmul
                    nc.tensor.matmul(zall_ps[:], eye16[0:1, h * 16:h * 16 + 16],
                                     zrow[:], start=(h == 0), stop=(h == 15),
                                     skip_group_check=True)
            zall = small.tile([16, 512], F32, tag="zall", bufs=1, name="zall")
            nc.vector.tensor_copy(out=zall[:], in_=zall_ps[:])
            nc.vector.reciprocal_approx_fast(out=zall[:], in_=zall[:])
            zinv_bf = small.tile([16, 512], BF, tag="zinvb", bufs=1,
                                 name="zinvb")
            nc.vector.tensor_copy(out=zinv_bf[:], in_=zall[:])
            for g in range(C8):
                zb = pp.tile([128, 512], F32, tag="mm", bufs=2, name="zb")
                nc.tensor.matmul(zb[:], esel[:, g, :], zinv_bf[:],
                                 start=True, stop=True)
                nc.vector.tensor_mul(oT[:, g, :], oT[:, g, :], zb[:])

        # ---------- load inputs ----------
        x1 = mp.tile([128, C8, T], F32R, tag="res", bufs=2, name="x1")
        nc.sync.dma_start(out=x1[:],
                          in_=dp["xT"].rearrange("p (c t) -> p c t", c=C8))
        kvT_r = dp["kvT"].rearrange("p (c t) -> p c t", c=C8)

        def kv_src(c, sl):
            t = mp.tile([128, 512], BF, tag="lnsrc", bufs=2, name="kvs")
            nc.sync.dma_start(out=t[:], in_=kvT_r[:, c, sl])
            return t[:]

        # PE warmup: keep TensorE busy during the initial input DMAs so the
        # HAM clock gate reaches 8/8 before real matmuls start.
        warm_ps = pp.tile([128, 128], F32, tag="ops", bufs=2, name="warm_ps")
        for _ in range(40):
            nc.tensor.matmul(warm_ps[:], ones_sb[:], ones_sb[:],
                             start=True, stop=True, skip_group_check=True)
        warm_out = small.tile([1, 128], F32, tag="zrow", bufs=1,
                              name="warm_out")
        nc.vector.tensor_copy(out=warm_out[:], in_=warm_ps[0:1, :])

        # ================= Phase 1: cross-attention =================
        q_in = mp.tile([128, C8, T], BF, tag="lnout", bufs=2, name="q_in")
        layer_norm(lambda c: x1[:, c, :], F32R, q_in, slice(0, T))

        qT1 = mp.tile([128, C8, T], BF, tag="qT", bufs=1, name="qT1")

        def ev_qT1(nt, ps):
            nc.vector.tensor_scalar_add(qT1[:, nt, :], ps[:],
                                        scalar1=bq_c[:, nt:nt + 1])

        wq_t = load_wchunk("wq", 0, 8, 1024)
        gemm(wq_t, lambda c: q_in[:, c, :], C8, ev_qT1)

        kT1 = mp.tile([128, C8, TF], BF, tag="kT", bufs=1, name="kT1")
        v1 = mp.tile([128, 8, H, 65], BF, tag="vv", bufs=1, name="v1")
        nc.vector.memset(v1[:, :, :, 64], 1.0)

        wk_t = load_wchunk("wkv", 0, 8, 1024)
        wv_t = load_wchunk("wkv", 1, 8, 1024)
        for th in range(2):
            sl = slice(th * T, (th + 1) * T)
            kv_in = mp.tile([128, C8, T], BF, tag="lnout", bufs=2,
                            name=f"kv_in{th}")
            layer_norm(lambda c, sl=sl: kv_src(c, sl), BF, kv_in, slice(0, T),
                       stag="ops")

            def ev_kT1(nt, ps, sl=sl):
                nc.vector.tensor_scalar_add(kT1[:, nt, sl], ps[:],
                                            scalar1=bk_c[:, nt:nt + 1])

            gemm(wk_t, lambda c: kv_in[:, c, :], C8, ev_kT1)
            build_v(wv_t, kv_in, range(th * 4, th * 4 + 4), v1, th * 4)

        oT1 = mp.tile([128, C8, T], BF, tag="oT", bufs=1, name="oT1")
        attention(qT1, kT1, v1, oT1)

        # x2 = x1 + Wco @ o + bco ; also exported as bf16 for the pair AG
        x2 = mp.tile([128, C8, T], F32R, tag="res", bufs=2, name="x2")

        def ev_x2(nt, ps):
            nc.vector.scalar_tensor_tensor(
                out=x2[:, nt, :], in0=ps[:], scalar=bco_c[:, nt:nt + 1],
                in1=x1[:, nt, :], op0=OP.add, op1=OP.add)

        wco_t = load_wchunk("wco", 0, 8, 1024)
        gemm(wco_t, lambda c: oT1[:, c, :], C8, ev_x2)
        # casting DMA (SWDGE): f32 residual -> bf16 AllGather payload
        nc.gpsimd.dma_start(out=x2bf_d[:],
                            in_=x2[:].rearrange("p c t -> p (c t)"))

        # ================= x2 exchange (pair AllGather) =================
        nc.gpsimd.collective_compute(
            "AllGather", OP.bypass,
            ins=[x2bf_d[:]],
            outs=[ag_out[:]],
            replica_groups=[[0, 1], [2, 3], [4, 5], [6, 7]])

        # ================= Phase 2: self-attention =================
        s_own = mp.tile([128, C8, T], BF, tag="lnout", bufs=2, name="s_own")
        layer_norm(lambda c: x2[:, c, :], F32R, s_own, slice(0, T))

        qT2 = mp.tile([128, C8, T], BF, tag="qT", bufs=1, name="qT2")
        kT2 = mp.tile([128, C8, TF], BF, tag="kT", bufs=1, name="kT2")
        v2 = mp.tile([128, 8, H, 65], BF, tag="vv", bufs=1, name="v2")
        nc.vector.memset(v2[:, :, :, 64], 1.0)

        def ev_qT2(nt, ps):
            nc.vector.tensor_scalar_add(qT2[:, nt, :], ps[:],
                                        scalar1=bqs_c[:, nt:nt + 1])

        wqs_t = load_wchunk("wqkv", 0, 8, 1024)
        gemm(wqs_t, lambda c: s_own[:, c, :], C8, ev_qT2)

        def ev_kT2_own(nt, ps):
            nc.vector.tensor_scalar_add(kT2[:, nt, 0:T], ps[:],
                                        scalar1=bks_c[:, nt:nt + 1])

        wks_t = load_wchunk("wqkv", 1, 8, 1024)
        gemm(wks_t, lambda c: s_own[:, c, :], C8, ev_kT2_own)
        wvs_t = load_wchunk("wqkv", 2, 8, 1024)
        build_v(wvs_t, s_own, range(0, 4), v2, 0)

        # remote half (depends on the AllGather)
        ag_rem = ag_out[bass.ds(rem_idx, 1), :, :].rearrange(
            "o p (c t) -> p (o c) t", c=C8)
        s_rem = mp.tile([128, C8, T], BF, tag="lnout", bufs=2, name="s_rem")

        def rem_src(c):
            t = mp.tile([128, 512], BF, tag="lnsrc", bufs=2, name="agrem")
            nc.sync.dma_start(out=t[:], in_=ag_rem[:, c, :])
            return t[:]

        layer_norm(rem_src, BF, s_rem, slice(0, T), stag="ops")

        def ev_kT2_rem(nt, ps):
            nc.vector.tensor_scalar_add(kT2[:, nt, T:TF], ps[:],
                                        scalar1=bks_c[:, nt:nt + 1])

        gemm(wks_t, lambda c: s_rem[:, c, :], C8, ev_kT2_rem)
        build_v(wvs_t, s_rem, range(4, 8), v2, 4)

        oT2 = mp.tile([128, C8, T], BF, tag="oT", bufs=1, name="oT2")
        attention(qT2, kT2, v2, oT2)

        x3 = mp.tile([128, C8, T], F32R, tag="res", bufs=2, name="x3")

        def ev_x3(nt, ps):
            nc.vector.scalar_tensor_tensor(
                out=x3[:, nt, :], in0=ps[:], scalar=bso_c[:, nt:nt + 1],
                in1=x2[:, nt, :], op0=OP.add, op1=OP.add)

        wso_t = load_wchunk("wso", 0, 8, 1024)
        gemm(wso_t, lambda c: oT2[:, c, :], C8, ev_x3)

        # ================= Phase 3: MLP =================
        m_in = mp.tile([128, C8, T], BF, tag="lnout", bufs=2, name="m_in")
        layer_norm(lambda c: x3[:, c, :], F32R, m_in, slice(0, T), stag="ops")

        hT = mp.tile([128, 32, T], BF, tag="hT", bufs=1, name="hT")

        def ev_h(ht, ps):
            nc.scalar.activation(out=hT[:, ht, :], in_=ps[:],
                                 func=AF.Gelu_apprx_tanh,
                                 bias=b1_c[:, ht:ht + 1], scale=1.0)

        for q in range(4):
            w1_t = load_wchunk("w1", q, 8, 1024)
            gemm(w1_t, lambda c: m_in[:, c, :], C8, ev_h, nt_off=q * C8)

        outT_r = dp["outT"].rearrange("p (c t) -> p c t", c=C8)
        for q in range(4):
            w2_t = load_wchunk("w2", q, 32, 256)
            for nt2 in range(2):
                nt = q * 2 + nt2
                ps = pp.tile([128, 512], F32, tag="mm", bufs=2, name="ops2")
                for kk in range(32):
                    nc.tensor.matmul(
                        ps[:], w2_t[:, kk, nt2 * 128:(nt2 + 1) * 128],
                        hT[:, kk, :], start=(kk == 0), stop=(kk == 31))
                ot = mp.tile([128, 512], F32, tag="otile", bufs=2, name="ot")
                nc.vector.tensor_scalar_add(ot[:], ps[:],
                                            scalar1=b2_c[:, nt:nt + 1])
                nc.sync.dma_start(out=outT_r[:, nt, :], in_=ot[:])


def _get_program():
    if "nc" not in _PROGRAM_CACHE:
        _PROGRAM_CACHE["nc"] = _build_program()
    return _PROGRAM_CACHE["nc"]


def _pack_weight(w, npc):
    """[Din, Dout] f32 -> [128, Q*CHUNK] bf16 with per-partition-contiguous
    2MB chunks of layout [chunk][cin][ncol]."""
    import ml_dtypes
    din, dout = w.shape
    c = din // 128
    q = dout // npc
    arr = np.ascontiguousarray(w).astype(ml_dtypes.bfloat16)
    arr = arr.reshape(c, 128, q, npc).transpose(1, 2, 0, 3)
    return np.ascontiguousarray(arr.reshape(128, q * c * npc))


def kernel(**inputs) -> np.ndarray:
    import ml_dtypes
    from concourse.bass_utils import run_bass_kernel_spmd

    nc = _get_program()

    f32 = lambda a: np.ascontiguousarray(np.asarray(a, np.float32))
    x = f32(inputs["x"])
    key_val = f32(inputs["key_val"])
    s1, b1n = f32(inputs["ln1_s"]), f32(inputs["ln1_b"])
    s2, b2n = f32(inputs["ln2_s"]), f32(inputs["ln2_b"])
    s3, b3n = f32(inputs["ln3_s"]), f32(inputs["ln3_b"])
    s4, b4n = f32(inputs["ln4_s"]), f32(inputs["ln4_b"])
    wq = f32(inputs["Wq"]) * s1[:, None]
    wkv = f32(inputs["Wkv"]) * s2[:, None]
    wqkv = f32(inputs["Wqkv"]) * s3[:, None]
    w1 = f32(inputs["W1"]) * s4[:, None]
    wco, wso, w2 = f32(inputs["Wco"]), f32(inputs["Wso"]), f32(inputs["W2"])
    bq = f32(inputs["Wq"]).T @ b1n
    bkv = f32(inputs["Wkv"]).T @ b2n
    bqkv = f32(inputs["Wqkv"]).T @ b3n
    b1f = f32(inputs["b1"]) + f32(inputs["W1"]).T @ b4n

    esel = np.zeros((16, C8 * 128), np.float32)
    for g in range(C8):
        esel[2 * g, g * 128:g * 128 + 64] = 1.0
        esel[2 * g + 1, g * 128 + 64:(g + 1) * 128] = 1.0

    bf = lambda a: np.ascontiguousarray(np.asarray(a, ml_dtypes.bfloat16))
    feat_major = lambda a, dt: np.ascontiguousarray(
        a.T.reshape(C8, 128, -1).transpose(1, 0, 2).reshape(128, -1)
        .astype(dt))

    shared = {
        "wq": _pack_weight(wq, 1024), "wkv": _pack_weight(wkv, 1024),
        "wqkv": _pack_weight(wqkv, 1024), "wco": _pack_weight(wco, 1024),
        "wso": _pack_weight(wso, 1024), "w1": _pack_weight(w1, 1024),
        "w2": _pack_weight(w2, 256),
        "bq": f32(bq)[None, :], "bk": f32(bkv[:D])[None, :],
        "bqs": f32(bqkv[:D])[None, :], "bks": f32(bqkv[D:2 * D])[None, :],
        "bco": f32(inputs["bco"] + wco.T @ bkv[D:])[None, :],
        "bso": f32(inputs["bso"] + wso.T @ bqkv[2 * D:])[None, :],
        "b1": f32(b1f)[None, :], "b2": f32(inputs["b2"])[None, :],
        "ones": np.ones((128, 128), np.float32),
        "ones_bf": bf(np.ones((128, 128))),
        "esel": bf(esel),
        "eye16": np.eye(16, dtype=np.float32).reshape(1, 256),
    }
    in_maps = []
    for c in range(N_CORES):
        b, s = c // 2, c % 2
        m = dict(shared)
        m["xT"] = feat_major(x[b, s * T:(s + 1) * T, :], np.float32)
        m["kvT"] = feat_major(key_val[b], ml_dtypes.bfloat16)
        in_maps.append(m)

    res = run_bass_kernel_spmd(nc, in_maps, list(range(N_CORES)))
    _PROGRAM_CACHE["last_result"] = res

    out = np.empty((B, NSEQ, D), np.float32)
    for c in range(N_CORES):
        b, s = c // 2, c % 2
        r = np.asarray(res.results[c]["outT"], np.float32)
        r = r.reshape(128, C8, T).transpose(1, 0, 2).reshape(D, T)
        out[b, s * T:(s + 1) * T, :] = r.T
    return out


# revision 23
# speedup vs baseline: 2.1931x; 1.0123x over previous
"""CrossAttnBlock kernel for 8 Trainium2 NeuronCores.

Sharding: core c -> (batch b = c//2, token-half s = c%2), 512 query tokens
per core. Cross-attention K/V is computed fully per core (duplicated within
the pair); after cross-attention the per-core residual x2 (bf16) is
exchanged with a pair-local 2-rank AllGather so each core rebuilds the
partner half's self-attn K/V locally (attention is permutation-invariant
over KV tokens, so own tokens always sit at positions 0:512).

All matmul operands are bf16 (fp32 PSUM accumulation); the residual stream
stays fp32. Weights are pre-shuffled on the host into [128, chunk*cin*ncol]
layout so every weight DMA is a single 2 MB transfer with 16 KB contiguous
per-partition lines. LayerNorm scale/bias are folded into the weights on the
host; the device computes only (x - mean) * rsqrt(var + eps), with
rsqrt obtained as exp(-0.5*ln(var+eps)) to stay inside one ACT table set.
K/V live entirely in SBUF. Softmax denominators come from a ones-column in
V; normalization uses reciprocal_approx_fast on all 16 heads at once plus a
selector-matrix matmul that broadcasts 1/Z to all 128 partitions per chunk.
"""
import sys

sys.path.insert(0, '/opt/trn_rl_repo')

import numpy as np
import concourse.bass as bass
from concourse import bacc
import concourse.tile as tile
from concourse import mybir

F32R = mybir.dt.float32r
F32 = mybir.dt.float32
BF = mybir.dt.bfloat16
AF = mybir.ActivationFunctionType
OP = mybir.AluOpType

N_CORES = 8
B, NSEQ, D, H, HD = 4, 1024, 1024, 16, 64
T = 512            # tokens owned per core
TF = 1024          # full token count per batch
C8 = D // 128      # feature chunks
CHUNK = 8 * 1024   # weight-chunk elems per partition (2 MB bf16 per chunk)
SCALE = 1.0 / float(np.sqrt(np.float32(HD)))
EPS = 1e-6

_PROGRAM_CACHE = {}

# weight name -> (n_chunks, cin_per_chunk, ncol_per_chunk)
W_SPECS = {
    "wq": (1, 8, 1024),
    "wkv": (2, 8, 1024),
    "wco": (1, 8, 1024),
    "wqkv": (3, 8, 1024),
    "wso": (1, 8, 1024),
    "w1": (4, 8, 1024),
    "w2": (4, 32, 256),
}


def _build_program():
    nc = bacc.Bacc("TRN2", target_bir_lowering=False, debug=False,
                   num_devices=N_CORES)

    dp = {}
    dp["xT"] = nc.declare_dram_parameter("xT", [128, C8 * T], F32R,
                                         isOutput=False)
    dp["kvT"] = nc.declare_dram_parameter("kvT", [128, C8 * TF], BF,
                                          isOutput=False)
    for nm, (q, _, _) in W_SPECS.items():
        dp[nm] = nc.declare_dram_parameter(nm, [128, q * CHUNK], BF,
                                           isOutput=False)
    # feature-major (column) biases
    for nm, n in [("bq", D), ("bk", D), ("bqs", D), ("bks", D),
                  ("bco", D), ("bso", D), ("b1", 4 * D), ("b2", D)]:
        dp[nm] = nc.declare_dram_parameter(nm, [1, n], F32, isOutput=False)
    dp["ones"] = nc.declare_dram_parameter("ones", [128, 128], F32R,
                                           isOutput=False)
    dp["ones_bf"] = nc.declare_dram_parameter("ones_bf", [128, 128], BF,
                                              isOutput=False)
    dp["esel"] = nc.declare_dram_parameter("esel", [16, C8 * 128], BF,
                                           isOutput=False)
    dp["eye16"] = nc.declare_dram_parameter("eye16", [1, 256], F32R,
                                            isOutput=False)
    dp["outT"] = nc.declare_dram_parameter("outT", [128, C8 * T], F32,
                                           isOutput=True)

    with tile.TileContext(nc) as tc:
        _emit(nc, tc, dp)
    nc.compile()
    return nc


def _emit(nc, tc, dp):
    import contextlib

    ctx = contextlib.ExitStack()
    with ctx:
        consts = ctx.enter_context(tc.tile_pool(name="consts", bufs=1))
        mp = ctx.enter_context(tc.tile_pool(name="main", bufs=1))
        pp = ctx.enter_context(tc.tile_pool(name="pp", bufs=1, space="PSUM"))
        small = ctx.enter_context(tc.tile_pool(name="small", bufs=1))
        dramp = ctx.enter_context(tc.tile_pool(name="dramp", bufs=1,
                                               space="DRAM"))

        # ---------- constants ----------
        ones_sb = consts.tile([128, 128], F32R)
        nc.sync.dma_start(out=ones_sb[:], in_=dp["ones"][:])
        ones_col = ones_sb[:, 0:1]
        ones_row = ones_sb[0:1, :]
        ones_bf = consts.tile([128, 128], BF)
        nc.sync.dma_start(out=ones_bf[:], in_=dp["ones_bf"][:])
        esel = consts.tile([16, C8, 128], BF)
        nc.sync.dma_start(out=esel[:],
                          in_=dp["esel"].rearrange("p (c n) -> p c n", c=C8))
        eye16 = consts.tile([1, 256], F32R)
        nc.sync.dma_start(out=eye16[:], in_=dp["eye16"][:])
        eps_t = consts.tile([1, 1], F32)
        nc.vector.memset(eps_t[:], EPS)

        def load_col(name, nchunk):
            col = consts.tile([128, nchunk], F32, name=f"col_{name}")
            nc.sync.dma_start(
                out=col[:], in_=dp[name].rearrange("o (c p) -> p (o c)", p=128))
            return col

        bq_c = load_col("bq", C8)
        bk_c = load_col("bk", C8)
        bqs_c = load_col("bqs", C8)
        bks_c = load_col("bks", C8)
        bco_c = load_col("bco", C8)
        bso_c = load_col("bso", C8)
        b1_c = load_col("b1", 32)
        b2_c = load_col("b2", C8)

        pid = nc.sync.partition_id()
        rem_idx = 1 - pid % 2

        # ---------- DRAM intermediates ----------
        x2bf_d = dramp.tile([128, C8 * T], BF, name="x2bf_d")
        ag_out = dramp.tile([2, 128, C8 * T], BF, name="ag_out")

        # ---------- helpers ----------
        def load_wchunk(name, q, cch, nch):
            t = mp.tile([128, cch, nch], BF, tag="wc", bufs=2,
                        name=f"w_{name}{q}")
            nc.sync.dma_start(
                out=t[:],
                in_=dp[name][:, q * CHUNK:(q + 1) * CHUNK].rearrange(
                    "p (c n) -> p c n", c=cch))
            return t

        def layer_norm(src_fn, src_dt, dst, dst_sl, stag="mm"):
            """dst[:, c, dst_sl] = (src - mean) * rsqrt(var + eps), bf16 out.

            src_fn(c) -> [128, 512] AP of dtype src_dt (F32R or BF).
            """
            ones_c = ones_col if src_dt == F32R else ones_bf[:, 0:1]
            stats_x = pp.tile([1, 512], F32, tag=stag, bufs=2, name="stats_x")
            stats_q = pp.tile([1, 512], F32, tag=stag, bufs=2, name="stats_q")
            for c in range(C8):
                xc = src_fn(c)
                sq = mp.tile([128, 512], F32R, tag="sq", bufs=2, name="sq")
                nc.vector.tensor_mul(sq[:], xc, xc)
                nc.tensor.matmul(stats_x[:], ones_c, xc,
                                 start=(c == 0), stop=(c == C8 - 1),
                                 skip_group_check=True)
                nc.tensor.matmul(stats_q[:], ones_col, sq[:],
                                 start=(c == 0), stop=(c == C8 - 1),
                                 skip_group_check=True)
            mean = small.tile([1, 512], F32R, tag="mean", bufs=1, name="mean")
            nc.vector.tensor_scalar_mul(mean[:], stats_x[:], 1.0 / D)
            var = small.tile([1, 512], F32R, tag="var", bufs=1, name="var")
            nc.vector.tensor_scalar(var[:], stats_q[:],
                                    scalar1=1.0 / D, scalar2=0.0,
                                    op0=OP.mult, op1=OP.add)
            m2 = small.tile([1, 512], F32R, tag="lntmp", bufs=1, name="m2")
            nc.vector.tensor_mul(m2[:], mean[:], mean[:])
            nc.vector.tensor_sub(var[:], var[:], m2[:])
            std = small.tile([1, 512], F32, tag="lntmp", bufs=1, name="std")
            nc.scalar.activation(out=std[:], in_=var[:], func=AF.Sqrt,
                                 bias=eps_t[:])
            nc.vector.reciprocal_approx_fast(out=std[:], in_=std[:])
            u = small.tile([1, 512], F32R, tag="u", bufs=1, name="u")
            nc.vector.tensor_copy(out=u[:], in_=std[:])
            v = small.tile([1, 512], F32R, tag="lntmp", bufs=1, name="v")
            nc.vector.tensor_mul(v[:], mean[:], u[:])
            a0 = pp.tile([128, 1024], F32, tag="sc", bufs=2, name="a0")
            nc.tensor.matmul(a0[:, 0:512], ones_row, u[:], start=True,
                             stop=True)
            c0 = pp.tile([128, 1024], F32, tag="sc", bufs=2, name="c0")
            nc.tensor.matmul(c0[:, 0:512], ones_row, v[:], start=True,
                             stop=True)
            if src_dt == F32R:
                a0r, c0r = a0[:, 0:512], c0[:, 0:512]
            else:
                a0s = mp.tile([128, 512], BF, tag="a0s", bufs=2, name="a0s")
                nc.scalar.activation(out=a0s[:], in_=a0[:, 0:512],
                                     func=AF.Copy)
                c0s = mp.tile([128, 512], BF, tag="a0s", bufs=2, name="c0s")
                nc.scalar.activation(out=c0s[:], in_=c0[:, 0:512],
                                     func=AF.Copy)
                a0r, c0r = a0s[:], c0s[:]
            for c in range(C8):
                xc = src_fn(c)
                tmp = mp.tile([128, 512], F32R, tag="lnt", bufs=2, name="lnt")
                nc.vector.tensor_mul(tmp[:], xc, a0r)
                nc.vector.tensor_sub(dst[:, c, dst_sl], tmp[:], c0r)

        def gemm(wt, rhs_fn, n_tiles, evict, nt_off=0):
            """Standard feature-major GEMM from one weight chunk.

            wt: [128, 8, 1024] bf16 chunk; rhs_fn(c) -> [128, 512] bf16 AP.
            evict(nt, ps) with nt in [nt_off, nt_off + n_tiles).
            """
            for nt in range(n_tiles):
                ps = pp.tile([128, 512], F32, tag="mm", bufs=2, name="gps")
                for c in range(C8):
                    nc.tensor.matmul(ps[:], wt[:, c, nt * 128:(nt + 1) * 128],
                                     rhs_fn(c), start=(c == 0),
                                     stop=(c == C8 - 1))
                evict(nt_off + nt, ps)

        def build_v(wt, src, jts, v_dst, jt_off):
            """Token-major V tiles for j-tiles jts, written into v_dst.

            wt: V weight chunk [128, 8, 1024]; src: [128, C8, 512] bf16;
            v_dst: [128, 8, 16, 65] bf16. jts index into v_dst; jt_off maps
            them onto src token positions. (V bias is folded into bco/bso
            on the host: softmax rows sum to 1.)
            """
            for dvh in range(2):
                for jt in jts:
                    ps = pp.tile([128, 512], F32, tag="mm", bufs=2, name="vps")
                    sl = slice((jt - jt_off) * 128, (jt - jt_off + 1) * 128)
                    for c in range(C8):
                        nc.tensor.matmul(ps[:], src[:, c, sl],
                                         wt[:, c, dvh * 512:(dvh + 1) * 512],
                                         start=(c == 0), stop=(c == C8 - 1))
                    nc.vector.tensor_copy(
                        out=v_dst[:, jt, dvh * 8:(dvh + 1) * 8, 0:64],
                        in_=ps[:].rearrange("p (h e) -> p h e", h=8))

        def attention(qT, kT, vv, oT):
            """oT (bf16, feature-major, unnormalized->normalized in place)."""
            zall_ps = pp.tile([16, 512], F32, tag="mm", bufs=2, name="zall_ps")
            for g in range(C8):  # head pair (2g, 2g+1)
                o_ps = []
                for hh in range(2):
                    o_ps.append(pp.tile([65, 512], F32, tag="ops", bufs=2,
                                        name="ops"))
                for jt2 in range(4):
                    pts = []
                    for hh in range(2):
                        off = hh * 64
                        s_ps = pp.tile([128, 1024], F32, tag="sc", bufs=2,
                                       name="sps")
                        for j in range(2):
                            jt = 2 * jt2 + j
                            nc.tensor.matmul(
                                s_ps[:, j * 512:(j + 1) * 512],
                                kT[off:off + 64, g, jt * 128:(jt + 1) * 128],
                                qT[off:off + 64, g, :],
                                start=True, stop=True)
                        pt = mp.tile([128, 1024], BF, tag="pt", bufs=3,
                                     name="pt")
                        nc.scalar.activation(out=pt[:], in_=s_ps[:],
                                             func=AF.Exp, scale=SCALE)
                        pts.append(pt)
                    for hh in range(2):
                        h = 2 * g + hh
                        for j in range(2):
                            jt = 2 * jt2 + j
                            nc.tensor.matmul(
                                o_ps[hh][:], vv[:, jt, h, :],
                                pts[hh][:, j * 512:(j + 1) * 512],
                                start=(jt == 0), stop=(jt == 7),
                                skip_group_check=True)
                for hh in range(2):
                    h = 2 * g + hh
                    off = hh * 64
                    nc.vector.tensor_copy(out=oT[off:off + 64, g, :],
                                          in_=o_ps[hh][0:64, :])
                    zrow = small.tile([1, 512], F32R, tag="zrow", bufs=1,
                                      name="zrow")
                    nc.vector.tensor_copy(out=zrow[:], in_=o_ps[hh][64:65, :])
                    # scatter Z_h into partition h of zall_ps via a K=1 matmul
                    nc.tensor.matmul(zall_ps[:], eye16[0:1, h * 16:h * 16 + 16],
                                     zrow[:], start=(h == 0), stop=(h == 15),
                                     skip_group_check=True)
            zall = small.tile([16, 512], F32, tag="zall", bufs=1, name="zall")
            nc.vector.tensor_copy(out=zall[:], in_=zall_ps[:])
            nc.vector.reciprocal_approx_fast(out=zall[:], in_=zall[:])
            zinv_bf = small.tile([16, 512], BF, tag="zinvb", bufs=1,
                                 name="zinvb")
            nc.vector.tensor_copy(out=zinv_bf[:], in_=zall[:])
            for g in range(C8):
                zb = pp.tile([128, 512], F32, tag="mm", bufs=2, name="zb")
                nc.tensor.matmul(zb[:], esel[:, g, :], zinv_bf[:],
                                 start=True, stop=True)
                nc.vector.tensor_mul(oT[:, g, :], oT[:, g, :], zb[:])

        # ---------- load inputs ----------
        x1 = mp.tile([128, C8, T], F32R, tag="res", bufs=2, name="x1")
        nc.sync.dma_start(out=x1[:],
                          in_=dp["xT"].rearrange("p (c t) -> p c t", c=C8))
        kvT_r = dp["kvT"].rearrange("p (c t) -> p c t", c=C8)

        def kv_src(c, sl):
            t = mp.tile([128, 512], BF, tag="lnsrc", bufs=2, name="kvs")
            nc.sync.dma_start(out=t[:], in_=kvT_r[:, c, sl])
            return t[:]

        # PE warmup: keep TensorE busy during the initial input DMAs so the
        # HAM clock gate reaches 8/8 before real matmuls start.
        warm_ps = pp.tile([128, 128], F32, tag="ops", bufs=2, name="warm_ps")
        for _ in range(96):
            nc.tensor.matmul(warm_ps[:], ones_sb[:], ones_sb[:],
                             start=True, stop=True, skip_group_check=True)
        warm_out = small.tile([1, 128], F32, tag="zrow", bufs=1,
                              name="warm_out")
        nc.vector.tensor_copy(out=warm_out[:], in_=warm_ps[0:1, :])

        # ================= Phase 1: cross-attention =================
        q_in = mp.tile([128, C8, T], BF, tag="lnout", bufs=2, name="q_in")
        layer_norm(lambda c: x1[:, c, :], F32R, q_in, slice(0, T))

        qT1 = mp.tile([128, C8, T], BF, tag="qT", bufs=1, name="qT1")

        def ev_qT1(nt, ps):
            nc.vector.tensor_scalar_add(qT1[:, nt, :], ps[:],
                                        scalar1=bq_c[:, nt:nt + 1])

        wq_t = load_wchunk("wq", 0, 8, 1024)
        gemm(wq_t, lambda c: q_in[:, c, :], C8, ev_qT1)

        kT1 = mp.tile([128, C8, TF], BF, tag="kT", bufs=1, name="kT1")
        v1 = mp.tile([128, 8, H, 65], BF, tag="vv", bufs=1, name="v1")
        nc.vector.memset(v1[:, :, :, 64], 1.0)

        wk_t = load_wchunk("wkv", 0, 8, 1024)
        wv_t = load_wchunk("wkv", 1, 8, 1024)
        for th in range(2):
            sl = slice(th * T, (th + 1) * T)
            kv_in = mp.tile([128, C8, T], BF, tag="lnout", bufs=2,
                            name=f"kv_in{th}")
            layer_norm(lambda c, sl=sl: kv_src(c, sl), BF, kv_in, slice(0, T),
                       stag="ops")

            def ev_kT1(nt, ps, sl=sl):
                nc.vector.tensor_scalar_add(kT1[:, nt, sl], ps[:],
                                            scalar1=bk_c[:, nt:nt + 1])

            gemm(wk_t, lambda c: kv_in[:, c, :], C8, ev_kT1)
            build_v(wv_t, kv_in, range(th * 4, th * 4 + 4), v1, th * 4)

        oT1 = mp.tile([128, C8, T], BF, tag="oT", bufs=1, name="oT1")
        attention(qT1, kT1, v1, oT1)

        # x2 = x1 + Wco @ o + bco ; also exported as bf16 for the pair AG
        x2 = mp.tile([128, C8, T], F32R, tag="res", bufs=2, name="x2")

        def ev_x2(nt, ps):
            nc.vector.scalar_tensor_tensor(
                out=x2[:, nt, :], in0=ps[:], scalar=bco_c[:, nt:nt + 1],
                in1=x1[:, nt, :], op0=OP.add, op1=OP.add)

        wco_t = load_wchunk("wco", 0, 8, 1024)
        gemm(wco_t, lambda c: oT1[:, c, :], C8, ev_x2)
        # casting DMA (SWDGE): f32 residual -> bf16 AllGather payload
        nc.gpsimd.dma_start(out=x2bf_d[:],
                            in_=x2[:].rearrange("p c t -> p (c t)"))

        # ================= x2 exchange (pair AllGather) =================
        nc.gpsimd.collective_compute(
            "AllGather", OP.bypass,
            ins=[x2bf_d[:]],
            outs=[ag_out[:]],
            replica_groups=[[0, 1], [2, 3], [4, 5], [6, 7]])

        # ================= Phase 2: self-attention =================
        s_own = mp.tile([128, C8, T], BF, tag="lnout", bufs=2, name="s_own")
        layer_norm(lambda c: x2[:, c, :], F32R, s_own, slice(0, T))

        qT2 = mp.tile([128, C8, T], BF, tag="qT", bufs=1, name="qT2")
        kT2 = mp.tile([128, C8, TF], BF, tag="kT", bufs=1, name="kT2")
        v2 = mp.tile([128, 8, H, 65], BF, tag="vv", bufs=1, name="v2")
        nc.vector.memset(v2[:, :, :, 64], 1.0)

        def ev_qT2(nt, ps):
            nc.vector.tensor_scalar_add(qT2[:, nt, :], ps[:],
                                        scalar1=bqs_c[:, nt:nt + 1])

        wqs_t = load_wchunk("wqkv", 0, 8, 1024)
        gemm(wqs_t, lambda c: s_own[:, c, :], C8, ev_qT2)

        def ev_kT2_own(nt, ps):
            nc.vector.tensor_scalar_add(kT2[:, nt, 0:T], ps[:],
                                        scalar1=bks_c[:, nt:nt + 1])

        wks_t = load_wchunk("wqkv", 1, 8, 1024)
        gemm(wks_t, lambda c: s_own[:, c, :], C8, ev_kT2_own)
        wvs_t = load_wchunk("wqkv", 2, 8, 1024)
        build_v(wvs_t, s_own, range(0, 4), v2, 0)

        # remote half (depends on the AllGather)
        ag_rem = ag_out[bass.ds(rem_idx, 1), :, :].rearrange(
            "o p (c t) -> p (o c) t", c=C8)
        s_rem = mp.tile([128, C8, T], BF, tag="lnout", bufs=2, name="s_rem")

        def rem_src(c):
            t = mp.tile([128, 512], BF, tag="lnsrc", bufs=2, name="agrem")
            nc.sync.dma_start(out=t[:], in_=ag_rem[:, c, :])
            return t[:]

        layer_norm(rem_src, BF, s_rem, slice(0, T), stag="ops")

        def ev_kT2_rem(nt, ps):
            nc.vector.tensor_scalar_add(kT2[:, nt, T:TF], ps[:],
                                        scalar1=bks_c[:, nt:nt + 1])

        gemm(wks_t, lambda c: s_rem[:, c, :], C8, ev_kT2_rem)
        build_v(wvs_t, s_rem, range(4, 8), v2, 4)

        oT2 = mp.tile([128, C8, T], BF, tag="oT", bufs=1, name="oT2")
        attention(qT2, kT2, v2, oT2)

        x3 = mp.tile([128, C8, T], F32R, tag="res", bufs=2, name="x3")

        def ev_x3(nt, ps):
            nc.vector.scalar_tensor_tensor(
                out=x3[:, nt, :], in0=ps[:], scalar=bso_c[:, nt:nt + 1],
                in1=x2[:, nt, :], op0=OP.add, op1=OP.add)

        wso_t = load_wchunk("wso", 0, 8, 1024)
        gemm(wso_t, lambda c: oT2[:, c, :], C8, ev_x3)

        # ================= Phase 3: MLP =================
        m_in = mp.tile([128, C8, T], BF, tag="lnout", bufs=2, name="m_in")
        layer_norm(lambda c: x3[:, c, :], F32R, m_in, slice(0, T), stag="ops")

        hT = mp.tile([128, 32, T], BF, tag="hT", bufs=1, name="hT")

        def ev_h(ht, ps):
            nc.scalar.activation(out=hT[:, ht, :], in_=ps[:],
                                 func=AF.Gelu_apprx_tanh,
                                 bias=b1_c[:, ht:ht + 1], scale=1.0)

        for q in range(4):
            w1_t = load_wchunk("w1", q, 8, 1024)
            gemm(w1_t, lambda c: m_in[:, c, :], C8, ev_h, nt_off=q * C8)

        outT_r = dp["outT"].rearrange("p (c t) -> p c t", c=C8)
        for q in range(4):
            w2_t = load_wchunk("w2", q, 32, 256)
            for nt2 in range(2):
                nt = q * 2 + nt2
                ps = pp.tile([128, 512], F32, tag="mm", bufs=2, name="ops2")
                for kk in range(32):
                    nc.tensor.matmul(
                        ps[:], w2_t[:, kk, nt2 * 128:(nt2 + 1) * 128],
                        hT[:, kk, :], start=(kk == 0), stop=(kk == 31))
                ot = mp.tile([128, 512], F32, tag="otile", bufs=2, name="ot")
                nc.vector.tensor_scalar_add(ot[:], ps[:],
                                            scalar1=b2_c[:, nt:nt + 1])
                nc.sync.dma_start(out=outT_r[:, nt, :], in_=ot[:])


def _get_program():
    if "nc" not in _PROGRAM_CACHE:
        _PROGRAM_CACHE["nc"] = _build_program()
    return _PROGRAM_CACHE["nc"]


def _pack_weight(w, npc):
    """[Din, Dout] f32 -> [128, Q*CHUNK] bf16 with per-partition-contiguous
    2MB chunks of layout [chunk][cin][ncol]."""
    import ml_dtypes
    din, dout = w.shape
    c = din // 128
    q = dout // npc
    arr = np.ascontiguousarray(w).astype(ml_dtypes.bfloat16)
    arr = arr.reshape(c, 128, q, npc).transpose(1, 2, 0, 3)
    return np.ascontiguousarray(arr.reshape(128, q * c * npc))


def kernel(**inputs) -> np.ndarray:
    import ml_dtypes
    from concourse.bass_utils import run_bass_kernel_spmd

    nc = _get_program()

    f32 = lambda a: np.ascontiguousarray(np.asarray(a, np.float32))
    x = f32(inputs["x"])
    key_val = f32(inputs["key_val"])
    s1, b1n = f32(inputs["ln1_s"]), f32(inputs["ln1_b"])
    s2, b2n = f32(inputs["ln2_s"]), f32(inputs["ln2_b"])
    s3, b3n = f32(inputs["ln3_s"]), f32(inputs["ln3_b"])
    s4, b4n = f32(inputs["ln4_s"]), f32(inputs["ln4_b"])
    wq = f32(inputs["Wq"]) * s1[:, None]
    wkv = f32(inputs["Wkv"]) * s2[:, None]
    wqkv = f32(inputs["Wqkv"]) * s3[:, None]
    w1 = f32(inputs["W1"]) * s4[:, None]
    wco, wso, w2 = f32(inputs["Wco"]), f32(inputs["Wso"]), f32(inputs["W2"])
    bq = f32(inputs["Wq"]).T @ b1n
    bkv = f32(inputs["Wkv"]).T @ b2n
    bqkv = f32(inputs["Wqkv"]).T @ b3n
    b1f = f32(inputs["b1"]) + f32(inputs["W1"]).T @ b4n

    esel = np.zeros((16, C8 * 128), np.float32)
    for g in range(C8):
        esel[2 * g, g * 128:g * 128 + 64] = 1.0
        esel[2 * g + 1, g * 128 + 64:(g + 1) * 128] = 1.0

    bf = lambda a: np.ascontiguousarray(np.asarray(a, ml_dtypes.bfloat16))
    feat_major = lambda a, dt: np.ascontiguousarray(
        a.T.reshape(C8, 128, -1).transpose(1, 0, 2).reshape(128, -1)
        .astype(dt))

    shared = {
        "wq": _pack_weight(wq, 1024), "wkv": _pack_weight(wkv, 1024),
        "wqkv": _pack_weight(wqkv, 1024), "wco": _pack_weight(wco, 1024),
        "wso": _pack_weight(wso, 1024), "w1": _pack_weight(w1, 1024),
        "w2": _pack_weight(w2, 256),
        "bq": f32(bq)[None, :], "bk": f32(bkv[:D])[None, :],
        "bqs": f32(bqkv[:D])[None, :], "bks": f32(bqkv[D:2 * D])[None, :],
        "bco": f32(inputs["bco"] + wco.T @ bkv[D:])[None, :],
        "bso": f32(inputs["bso"] + wso.T @ bqkv[2 * D:])[None, :],
        "b1": f32(b1f)[None, :], "b2": f32(inputs["b2"])[None, :],
        "ones": np.ones((128, 128), np.float32),
        "ones_bf": bf(np.ones((128, 128))),
        "esel": bf(esel),
        "eye16": np.eye(16, dtype=np.float32).reshape(1, 256),
    }
    in_maps = []
    for c in range(N_CORES):
        b, s = c // 2, c % 2
        m = dict(shared)
        m["xT"] = feat_major(x[b, s * T:(s + 1) * T, :], np.float32)
        m["kvT"] = feat_major(key_val[b], ml_dtypes.bfloat16)
        in_maps.append(m)

    res = run_bass_kernel_spmd(nc, in_maps, list(range(N_CORES)))
    _PROGRAM_CACHE["last_result"] = res

    out = np.empty((B, NSEQ, D), np.float32)
    for c in range(N_CORES):
        b, s = c // 2, c % 2
        r = np.asarray(res.results[c]["outT"], np.float32)
        r = r.reshape(128, C8, T).transpose(1, 0, 2).reshape(D, T)
        out[b, s * T:(s + 1) * T, :] = r.T
    return out
